# revision 9
# baseline (speedup 1.0000x reference)
"""GNN message-passing kernel for Trainium2 (Bass/Tile), 8-core SPMD.

Strategy (v2 — sharded dense):
- Node tensors live TRANSPOSED on chip: xT [128 features (partitions), nodes].
- Nodes padded 40000 -> 40960 (320 blocks of 128); 8 cores; core r owns
  node shard [r*5120, (r+1)*5120) for BOTH the dense pipeline and the
  edge aggregation (edges assigned to the core owning their target col).
- Dense (x@W, BN, relu) is sharded: each core processes only its 5120
  nodes; BN batch stats are partial sums AllReduce'd as tiny [128,K]
  vectors (pad columns are kept exactly zero via a mask so masked sums
  equal sums over real nodes).
- Neighbor aggregation: h = x@Wnb computed for own shard in node-major
  PE blocks, written to DRAM and AllGather'd into a full h-table
  (node rows permuted as r*5120 + (n%5120%128)*40 + (n%5120//128) so the
  h-pass store is one contiguous DMA); per-edge rows fetched with
  dma_gather (int16 indices, two halves of the permuted node space) and
  scatter-added with PE one-hot matmuls (fp8 one-hot streamed from DRAM,
  built once on host from edge_index).
- Edge branch: ea = segment_sum(edge_attr@Wedge + bedge, row) factored as
  Wedge (x) s + bedge (x) c_out with s = segment_sum(edge_attr, row)
  computed on device by a row-sorted one-hot matmul pass (local to the
  shard; no collective needed); degree counts from host bincount.
  Biases followed by BN cancel inside BN and are dropped; bnb enters via
  c_in (x) bnb added to aggr before its BN.
"""
import numpy as np
import ml_dtypes

F = 128
L = 3
EPS = 1e-5
NREAL = 40000
NE = 640000
NCORES = 8
NP_ = 40960
SHARD = NP_ // NCORES       # 5120
HALF = NP_ // 2             # 20480
NBLK = SHARD // F           # 40 blocks per core
CHUNK = 512                 # dense free-dim chunk
NCH_SH = SHARD // CHUNK     # 10 dense chunks per core

BF16 = np.float16
FP8 = ml_dtypes.float8_e4m3


def _ceil(a, b):
    return -(-a // b)


def _wrap_idx16(vals):
    """int16 gather-index layout: value j at [j%16, j//16], tiled to 128 parts."""
    n = vals.shape[0]
    a = vals.reshape(n // 16, 16).T.astype(np.int16)   # [16, n/16]
    return np.tile(a, (8, 1))                          # [128, n/16]


def _prep(node_attr, edge_index, edge_attr):
    """Host-side index preprocessing -> per-core arrays + metadata."""
    row = edge_index[0].astype(np.int64)
    col = edge_index[1].astype(np.int64)
    ea = edge_attr[:, 0].astype(np.float32)

    # permuted h-table row index: node n -> (n//SHARD)*SHARD + (n%SHARD%F)*NBLK
    # + (n%SHARD//F), so that core r's h-pass writes its table slice with one
    # contiguous [128, SHARD] DMA (partition-major within the shard).
    rl = row % SHARD
    perm = (row // SHARD) * SHARD + (rl % F) * NBLK + (rl // F)

    shard = col // SHARD
    half = perm // HALF
    blk = (col % SHARD) // F
    tloc = col % F

    # --- col pass (neighbor aggregation) ---
    cnt = np.zeros((NCORES, 2, NBLK), np.int64)
    np.add.at(cnt, (shard, half, blk), 1)
    sseg = _ceil(max(int(cnt.max()), 1), F) * F
    cpb = sseg // F                   # chunks per (half, block)
    nch_h = NBLK * cpb                # chunks per half
    nstream_h = NBLK * sseg           # slots per half
    order = np.lexsort((blk, half, shard))
    so_shard, so_half, so_blk = shard[order], half[order], blk[order]
    grp = ((so_shard * 2 + so_half) * NBLK + so_blk)
    grp_start = np.zeros(NCORES * 2 * NBLK + 1, np.int64)
    np.add.at(grp_start, grp + 1, 1)
    grp_start = np.cumsum(grp_start)
    within = np.arange(NE) - grp_start[grp]
    slot = (so_half * NBLK + so_blk) * sseg + within   # slot in core stream

    gsrc = np.zeros((NCORES, 2 * nstream_h), np.int16)
    gsrc[so_shard, slot] = (perm[order] - so_half * HALF).astype(np.int16)
    oh_col = np.zeros((NCORES, 2 * nstream_h, F), np.uint8)
    oh_col[so_shard, slot, tloc[order]] = 1

    gidx = np.stack([_wrap_idx16(gsrc[r]) for r in range(NCORES)])  # [8,128,S/16]
    nch = 2 * nch_h
    g2 = 2 * cpb                      # one-hot chunks per DMA group
    # one-hot: [ngrp, 128 part, g2 chunk, 128] fp8
    ohc = oh_col.reshape(NCORES, nch // g2, g2, F, F).transpose(0, 1, 3, 2, 4)
    ohc = np.ascontiguousarray(ohc).astype(FP8)

    # --- row pass (edge branch s = segment_sum(edge_attr, row)) ---
    rshard = row // SHARD
    rblk = (row % SHARD) // F
    rloc = row % F
    rcnt = np.zeros((NCORES, NBLK), np.int64)
    np.add.at(rcnt, (rshard, rblk), 1)
    rseg = _ceil(max(int(rcnt.max()), 1), F) * F
    cpb2 = rseg // F
    nch2 = NBLK * cpb2
    rorder = np.lexsort((rblk, rshard))
    ro_shard, ro_blk = rshard[rorder], rblk[rorder]
    rgrp = ro_shard * NBLK + ro_blk
    rgs = np.zeros(NCORES * NBLK + 1, np.int64)
    np.add.at(rgs, rgrp + 1, 1)
    rgs = np.cumsum(rgs)
    rwithin = np.arange(NE) - rgs[rgrp]
    rslot = ro_blk * rseg + rwithin

    eav = np.zeros((NCORES, NBLK * rseg), np.float32)
    eav[ro_shard, rslot] = ea[rorder]
    oh_row = np.zeros((NCORES, NBLK * rseg, F), np.uint8)
    oh_row[ro_shard, rslot, rloc[rorder]] = 1
    ohr = oh_row.reshape(NCORES, nch2 // 8, 8, F, F).transpose(0, 1, 3, 2, 4)
    ohr = np.ascontiguousarray(ohr).astype(FP8)
    eav_t = np.ascontiguousarray(
        eav.reshape(NCORES, nch2, F).transpose(0, 2, 1)).astype(BF16)

    # degree counts (pure edge_index metadata), per-core slices
    c_out = np.bincount(row, minlength=NP_).astype(np.float32)
    c_in = np.bincount(col, minlength=NP_).astype(np.float32)
    cot_r = np.ascontiguousarray(c_out.reshape(NCORES, 1, SHARD))
    cin_r = np.ascontiguousarray(c_in.reshape(NCORES, 1, SHARD))

    naT = np.zeros((2, NP_), np.float32)
    naT[:, :NREAL] = node_attr.T
    naT_r = np.ascontiguousarray(
        naT.reshape(2, NCORES, SHARD).transpose(1, 0, 2)).astype(BF16)

    maskv = np.zeros(NP_, np.float32)
    maskv[:NREAL] = 1.0
    mask_r = np.ascontiguousarray(np.broadcast_to(
        maskv.reshape(NCORES, 1, SHARD), (NCORES, F, SHARD))
        .transpose(0, 1, 2)).astype(BF16)
    mask_r = np.ascontiguousarray(mask_r)

    return dict(sseg=sseg, cpb=cpb, nch=nch, rseg=rseg, cpb2=cpb2, nch2=nch2,
                gidx=gidx, ohc=ohc, ohr=ohr, eav_t=eav_t,
                cot_r=cot_r, cin_r=cin_r, naT_r=naT_r, mask_r=mask_r)


def _build(meta):
    """Build the Bass program. Returns nc."""
    import concourse.bass as bass
    import concourse.tile as tile
    from concourse import bacc, mybir

    cpb, nch, cpb2, nch2 = meta["cpb"], meta["nch"], meta["cpb2"], meta["nch2"]
    nch_h = nch // 2
    nstream_h = nch_h * F
    g2 = 2 * cpb                    # one-hot chunks per DMA group
    GCALL = 4 * cpb * F             # idxs per gather call -> 10 calls per half
    ncalls_h = nstream_h // GCALL
    dt = mybir.dt
    AX = mybir.AxisListType.X
    OP = mybir.AluOpType
    ACTF = mybir.ActivationFunctionType

    nc = bacc.Bacc("TRN2", target_bir_lowering=False, debug=False,
                   num_devices=NCORES, num_swdge_queues=4)

    # ---- DRAM tensors ----
    def din(name, shape, d):
        return nc.dram_tensor(name, shape, d, kind="ExternalInput")

    naT = din("naT", [2, SHARD], dt.float16)
    gidx = din("gidx", [128, 2 * nstream_h // 16], dt.int16)
    ohc = din("ohc", [nch // g2, 128, g2, F], dt.float8e4)
    ohr = din("ohr", [nch2 // 8, 128, 8, F], dt.float8e4)
    eav = din("eav", [128, nch2], dt.float16)
    cot = din("cot", [1, SHARD], dt.float32)
    cin = din("cin", [1, SHARD], dt.float32)
    maskb = din("maskb", [F, SHARD], dt.float16)
    W0 = din("W0", [2, F], dt.float16)
    Wn = [din(f"Wn{i}", [F, F], dt.float16) for i in range(L)]
    Wb = [din(f"Wb{i}", [F, F], dt.float16) for i in range(L)]
    W1 = [din(f"W1{i}", [F, F], dt.float16) for i in range(L)]
    W2 = [din(f"W2{i}", [F, F], dt.float16) for i in range(L)]
    Wec = [din(f"Wec{i}", [2, F], dt.float32) for i in range(L)]
    Bnb = [din(f"Bnb{i}", [1, F], dt.float32) for i in range(L)]
    gcol = {}
    for nm in ("g0", "bt0"):
        gcol[nm] = din(nm, [F, 1], dt.float32)
    for i in range(L):
        for nm in ("gn", "btn", "ge", "bte", "gnb", "btnb",
                   "gm1", "btm1", "gm2", "btm2"):
            gcol[f"{nm}{i}"] = din(f"{nm}{i}", [F, 1], dt.float32)

    out = nc.dram_tensor("out", [F, SHARD], dt.float32, kind="ExternalOutput")

    with tile.TileContext(nc) as tc:
        import contextlib
        ctx = contextlib.ExitStack()
        with ctx:
            sb = ctx.enter_context(tc.tile_pool(name="sb", bufs=1))
            wpool = ctx.enter_context(tc.tile_pool(name="wp", bufs=1))
            tp = ctx.enter_context(tc.tile_pool(name="tp", bufs=2))
            gp = ctx.enter_context(tc.tile_pool(name="gp", bufs=3))
            ohp = ctx.enter_context(tc.tile_pool(name="ohp", bufs=3))
            ohrp = ctx.enter_context(tc.tile_pool(name="ohrp", bufs=2))
            hst = ctx.enter_context(tc.tile_pool(name="hst", bufs=1))
            stp = ctx.enter_context(tc.tile_pool(name="stp", bufs=1))
            afp = ctx.enter_context(tc.tile_pool(name="afp", bufs=6))
            ps_mm = ctx.enter_context(tc.tile_pool(name="psmm", bufs=2, space="PSUM"))
            ps_sc = ctx.enter_context(tc.tile_pool(name="pssc", bufs=2, space="PSUM"))
            ps_ou = ctx.enter_context(tc.tile_pool(name="psou", bufs=2, space="PSUM"))
            ps_sp = ctx.enter_context(tc.tile_pool(name="pssp", bufs=2, space="PSUM"))
            dram = ctx.enter_context(tc.tile_pool(name="dram", bufs=1, space="DRAM"))

            # persistent SBUF
            x_sh = sb.tile([F, SHARD], dt.float16)
            xn_sh = sb.tile([F, SHARD], dt.float16)
            ea_sh = sb.tile([F, SHARD], dt.float16)
            aggr_sh = sb.tile([F, SHARD], dt.float32)
            mask_sb = sb.tile([F, SHARD], dt.float16)
            nc.sync.dma_start(mask_sb[:], maskb.ap())
            naT_sb = sb.tile([2, SHARD], dt.float16)
            nc.sync.dma_start(naT_sb[:], naT.ap())
            scs = sb.tile([2, SHARD], dt.float32)   # [s ; c_out]
            nc.sync.dma_start(scs[1:2, :], cot.ap())
            cii = sb.tile([1, SHARD], dt.float32)
            nc.sync.dma_start(cii[:], cin.ap())
            gidx_sb = sb.tile([128, 2 * nstream_h // 16], dt.int16)
            nc.sync.dma_start(gidx_sb[:], gidx.ap())
            eav_sb = sb.tile([128, nch2], dt.float16)
            nc.sync.dma_start(eav_sb[:], eav.ap())

            # weights resident
            W0_sb = wpool.tile([2, F], dt.float16)
            nc.sync.dma_start(W0_sb[:], W0.ap())
            Wn_sb, Wb_sb, W1_sb, W2_sb, Wec_sb, Bnb_sb = [], [], [], [], [], []
            for i in range(L):
                for lst, t_ in ((Wn_sb, Wn[i]), (Wb_sb, Wb[i]),
                                (W1_sb, W1[i]), (W2_sb, W2[i])):
                    w = wpool.tile([F, F], dt.float16, tag=f"w{len(lst)}_{t_.name}")
                    nc.sync.dma_start(w[:], t_.ap())
                    lst.append(w)
                w = wpool.tile([2, F], dt.float32, tag=f"wec{i}")
                nc.sync.dma_start(w[:], Wec[i].ap())
                Wec_sb.append(w)
                w = wpool.tile([1, F], dt.float32, tag=f"bnb{i}")
                nc.sync.dma_start(w[:], Bnb[i].ap())
                Bnb_sb.append(w)
            gc_sb = {}
            for nm, t_ in gcol.items():
                w = wpool.tile([F, 1], dt.float32, tag=f"p{nm}")
                nc.sync.dma_start(w[:], t_.ap())
                gc_sb[nm] = w

            # DRAM scratch
            agh_ins = [dram.tile([128, SHARD], dt.float16, tag=f"ahi{i}",
                                 name=f"ahi{i}") for i in range(L)]
            agh_outs = [dram.tile([2, HALF, F], dt.float16, addr_space="Shared",
                                  tag=f"aho{i}", name=f"aho{i}")
                        for i in range(L)]
            htab = dram.tile([2, HALF, F], dt.float16, tag="htab", name="htab")
            st_ins, st_outs = [], []
            for k, w_ in enumerate([2] + [6, 2, 2] * L):
                st_ins.append(dram.tile([F, w_], dt.float32, tag=f"sti{k}",
                                        name=f"sti{k}"))
                st_outs.append(dram.tile([F, w_], dt.float32,
                                         tag=f"sto{k}", name=f"sto{k}"))

            # ---------- helpers ----------
            def finalize_bn(g, bt, ssum, ssq):
                """column affine from global sums: (scale, shift) [128,1] f32"""
                mean = afp.tile([F, 1], dt.float32)
                nc.vector.tensor_scalar_mul(mean[:], ssum, 1.0 / NREAL)
                m2t = afp.tile([F, 1], dt.float32)
                nc.scalar.activation(m2t[:], mean[:], ACTF.Square)
                var = afp.tile([F, 1], dt.float32)
                nc.vector.scalar_tensor_tensor(
                    out=var[:], in0=ssq, scalar=1.0 / NREAL, in1=m2t[:],
                    op0=OP.mult, op1=OP.subtract)
                nc.vector.tensor_scalar_add(var[:], var[:], EPS)
                lnv = afp.tile([F, 1], dt.float32)
                nc.scalar.activation(lnv[:], var[:], ACTF.Ln)
                isig = afp.tile([F, 1], dt.float32)
                nc.scalar.activation(isig[:], lnv[:], ACTF.Exp, scale=-0.5)
                scale = afp.tile([F, 1], dt.float32)
                nc.vector.tensor_mul(scale[:], g[:], isig[:])
                nscale = afp.tile([F, 1], dt.float32)
                nc.vector.tensor_scalar_mul(nscale[:], scale[:], -1.0)
                shift = afp.tile([F, 1], dt.float32)
                nc.vector.scalar_tensor_tensor(
                    out=shift[:], in0=mean[:], scalar=nscale[:], in1=bt[:],
                    op0=OP.mult, op1=OP.add)
                return scale, shift

            def stat_pass(src, c, ssum_sl, ssq_sl):
                """accumulate sum / sumsq of one [F, CHUNK] chunk into slot c"""
                nc.vector.tensor_reduce(ssum_sl[:, c:c + 1], src, AX, OP.add)
                tr = tp.tile([F, CHUNK], dt.float32, tag="tr")
                nc.scalar.activation(tr[:], src, ACTF.Square,
                                     accum_out=ssq_sl[:, c:c + 1])

            def slot_reduce(slots, n):
                r = afp.tile([F, 1], dt.float32)
                nc.vector.tensor_reduce(r[:], slots[:, :n], AX, OP.add)
                return r

            def do_allreduce(k, cols):
                """cols: list of [F,1] tiles -> AllReduce -> [F, len] tile"""
                w_ = len(cols)
                stc = stp.tile([F, 8], dt.float32, tag=f"stc{k}")
                for j, cl in enumerate(cols):
                    nc.vector.tensor_copy(stc[:, j:j + 1], cl[:])
                nc.sync.dma_start(st_ins[k][:, :], stc[:, :w_])
                nc.gpsimd.collective_compute(
                    "AllReduce", OP.add, replica_groups=[list(range(NCORES))],
                    ins=[st_ins[k].opt()], outs=[st_outs[k].opt()])
                stg = stp.tile([F, 8], dt.float32, tag=f"stg{k}")
                nc.sync.dma_start(stg[:, :w_], st_outs[k][:, :])
                return stg

            # ---------- layer 0: x0 = relu(bn0(naT @ W0)) * mask ----------
            ss0 = stp.tile([F, NCH_SH], dt.float32, tag="ss0")
            sq0 = stp.tile([F, NCH_SH], dt.float32, tag="sq0")
            for c in range(NCH_SH):
                sl = slice(c * CHUNK, (c + 1) * CHUNK)
                ps = ps_mm.tile([F, CHUNK], dt.float32, tag="mm")
                nc.tensor.matmul(ps[:], lhsT=W0_sb[:], rhs=naT_sb[:, sl],
                                 start=True, stop=True)
                stat_pass(ps[:], c, ss0, sq0)
                nc.scalar.activation(x_sh[:, sl], ps[:], ACTF.Copy)
            stg = do_allreduce(0, [slot_reduce(ss0, NCH_SH),
                                   slot_reduce(sq0, NCH_SH)])
            sc0, sh0 = finalize_bn(gc_sb["g0"], gc_sb["bt0"],
                                   stg[:, 0:1], stg[:, 1:2])
            for c in range(NCH_SH):
                sl = slice(c * CHUNK, (c + 1) * CHUNK)
                nc.scalar.activation(x_sh[:, sl], x_sh[:, sl], ACTF.Relu,
                                     bias=sh0[:], scale=sc0[:])
                nc.vector.tensor_mul(x_sh[:, sl], x_sh[:, sl], mask_sb[:, sl])

            # ---------- s pass: s = segment_sum(edge_attr, row), local ----------
            for b in range(NBLK):
                pss = ps_sp.tile([1, F], dt.float32, tag="sp")
                for k in range(cpb2):
                    ci = b * cpb2 + k
                    g8 = ci // 8
                    if ci % 8 == 0:
                        ohrt = ohrp.tile([128, 8, F], dt.float8e4, tag="ohr")
                        nc.sync.dma_start(ohrt[:], ohr.ap()[g8])
                    nc.tensor.matmul(pss[:], lhsT=eav_sb[:, ci:ci + 1],
                                     rhs=ohrt[:, ci % 8, :],
                                     start=(k == 0), stop=(k == cpb2 - 1))
                nc.vector.tensor_copy(scs[0:1, b * F:(b + 1) * F], pss[:])

            # ---------- layers ----------
            gather_seq = [0]
            for i in range(L):
                # h-pass: node-major h = x @ Wb[i] for own shard -> one DMA
                hs = hst.tile([128, SHARD], dt.float16, tag="hs")
                for b in range(NBLK):
                    ps = ps_sc.tile([F, F], dt.float32, tag="sc")
                    nc.tensor.matmul(ps[:], lhsT=x_sh[:, b * F:(b + 1) * F],
                                     rhs=Wb_sb[i][:], start=True, stop=True)
                    nc.vector.tensor_copy(hs[:, b * F:(b + 1) * F], ps[:])
                nc.sync.dma_start(agh_ins[i][:, :], hs[:])
                nc.gpsimd.collective_compute(
                    "AllGather", OP.bypass, replica_groups=[list(range(NCORES))],
                    ins=[agh_ins[i].opt()], outs=[agh_outs[i].opt()])
                # bounce Shared AllGather output to a private table: dma_gather
                # from Shared scratchpad is untested on HW
                for h in range(2):
                    nc.sync.dma_start(htab[h], agh_outs[i][h])

                # xn pass: xn = x @ Wn[i] (stored) + stats
                ssn = stp.tile([F, NCH_SH], dt.float32, tag="ssn")
                sqn = stp.tile([F, NCH_SH], dt.float32, tag="sqn")
                sse = stp.tile([F, NCH_SH], dt.float32, tag="sse")
                sqe = stp.tile([F, NCH_SH], dt.float32, tag="sqe")
                for c in range(NCH_SH):
                    sl = slice(c * CHUNK, (c + 1) * CHUNK)
                    ps = ps_mm.tile([F, CHUNK], dt.float32, tag="mm")
                    nc.tensor.matmul(ps[:], lhsT=Wn_sb[i][:], rhs=x_sh[:, sl],
                                     start=True, stop=True)
                    stat_pass(ps[:], c, ssn, sqn)
                    nc.vector.tensor_copy(xn_sh[:, sl], ps[:])
                    # ea chunk: rank-2 [Wedge;bedge] x [s;c_out]
                    pse = ps_ou.tile([F, CHUNK], dt.float32, tag="pse")
                    nc.tensor.matmul(pse[:], lhsT=Wec_sb[i][:], rhs=scs[:, sl],
                                     start=True, stop=True)
                    stat_pass(pse[:], c, sse, sqe)
                    nc.scalar.activation(ea_sh[:, sl], pse[:], ACTF.Copy)

                # scatter: per half, gather calls + one-hot matmuls -> aggr
                ssa = stp.tile([F, NBLK], dt.float32, tag="ssa")
                sqa = stp.tile([F, NBLK], dt.float32, tag="sqa")
                for h in range(2):
                    base = h * nstream_h
                    for call in range(ncalls_h):
                        gt = gp.tile([128, 4 * cpb, F], dt.float16, tag="g")
                        j0 = base + call * GCALL
                        # queue from a global counter: tile rotates DMASW sem
                        # lanes mod 8 per pool-DMA inst; lane L must always
                        # pair with queue L%4
                        nc.gpsimd.dma_gather(
                            out_ap=gt[:],
                            in_ap=htab[h],
                            idxs_ap=gidx_sb[:, j0 // 16:(j0 + GCALL) // 16],
                            num_idxs=GCALL, num_idxs_reg=GCALL, elem_size=F,
                            queue_num=gather_seq[0] % 4,
                            single_packet=False)
                        gather_seq[0] += 1
                        for k8 in range(4 * cpb):
                            ci = call * 4 * cpb + k8    # chunk within half
                            gci = base // F + ci        # global chunk
                            if gci % g2 == 0:
                                oht = ohp.tile([128, g2, F], dt.float8e4,
                                               tag="oh")
                                nc.sync.dma_start(oht[:], ohc.ap()[gci // g2])
                            b = ci // cpb
                            k = ci % cpb
                            if k == 0:
                                psb = ps_sc.tile([F, F], dt.float32, tag="sc")
                            last = (k == cpb - 1)
                            nc.tensor.matmul(psb[:], lhsT=gt[:, k8, :],
                                             rhs=oht[:, gci % g2, :],
                                             start=(k == 0),
                                             stop=(last and h == 0))
                            if last:
                                dst = aggr_sh[:, b * F:(b + 1) * F]
                                if h == 0:
                                    nc.scalar.activation(dst, psb[:], ACTF.Copy)
                                else:
                                    # fold bnb (x) c_in into the same psum
                                    nc.tensor.matmul(
                                        psb[:], lhsT=Bnb_sb[i][:],
                                        rhs=cii[:, b * F:(b + 1) * F],
                                        start=False, stop=True)
                                    # aggr += psb ; stats
                                    nc.vector.scalar_tensor_tensor(
                                        out=dst, in0=psb[:], scalar=1.0,
                                        in1=dst, op0=OP.mult, op1=OP.add)
                                    nc.vector.tensor_reduce(
                                        ssa[:, b:b + 1], dst, AX, OP.add)
                                    tr8 = tp.tile([F, F], dt.float32, tag="tr8")
                                    nc.scalar.activation(
                                        tr8[:], dst, ACTF.Square,
                                        accum_out=sqa[:, b:b + 1])

                # AR#1: xn / ea / aggr stats
                stg = do_allreduce(1 + 3 * i, [
                    slot_reduce(ssn, NCH_SH), slot_reduce(sqn, NCH_SH),
                    slot_reduce(sse, NCH_SH), slot_reduce(sqe, NCH_SH),
                    slot_reduce(ssa, NBLK), slot_reduce(sqa, NBLK)])
                sc_n, sh_n = finalize_bn(gc_sb[f"gn{i}"], gc_sb[f"btn{i}"],
                                         stg[:, 0:1], stg[:, 1:2])
                sc_e, sh_e = finalize_bn(gc_sb[f"ge{i}"], gc_sb[f"bte{i}"],
                                         stg[:, 2:3], stg[:, 3:4])
                sc_a, sh_a = finalize_bn(gc_sb[f"gnb{i}"], gc_sb[f"btnb{i}"],
                                         stg[:, 4:5], stg[:, 5:6])
                shsum = afp.tile([F, 1], dt.float32)
                nc.vector.scalar_tensor_tensor(
                    out=shsum[:], in0=sh_n[:], scalar=1.0, in1=sh_e[:],
                    op0=OP.mult, op1=OP.add)
                nc.vector.scalar_tensor_tensor(
                    out=shsum[:], in0=shsum[:], scalar=1.0, in1=sh_a[:],
                    op0=OP.mult, op1=OP.add)

                # y1 = relu(sc_n*xn + sc_a*aggr + sc_e*ea + shsum) * mask ; m1
                ss1 = stp.tile([F, NCH_SH], dt.float32, tag="ss1")
                sq1 = stp.tile([F, NCH_SH], dt.float32, tag="sq1")
                for c in range(NCH_SH):
                    sl = slice(c * CHUNK, (c + 1) * CHUNK)
                    u2 = tp.tile([F, CHUNK], dt.float32, tag="u2")
                    nc.vector.tensor_scalar_mul(u2[:], xn_sh[:, sl], sc_n[:])
                    nc.vector.scalar_tensor_tensor(
                        out=u2[:], in0=aggr_sh[:, sl], scalar=sc_a[:],
                        in1=u2[:], op0=OP.mult, op1=OP.add)
                    nc.vector.scalar_tensor_tensor(
                        out=u2[:], in0=ea_sh[:, sl], scalar=sc_e[:],
                        in1=u2[:], op0=OP.mult, op1=OP.add)
                    y1 = tp.tile([F, CHUNK], dt.float16, tag="y1")
                    nc.scalar.activation(y1[:], u2[:], ACTF.Relu,
                                         bias=shsum[:], scale=1.0)
                    nc.vector.tensor_mul(y1[:], y1[:], mask_sb[:, sl])
                    ps1 = ps_mm.tile([F, CHUNK], dt.float32, tag="mm")
                    nc.tensor.matmul(ps1[:], lhsT=W1_sb[i][:], rhs=y1[:],
                                     start=True, stop=True)
                    stat_pass(ps1[:], c, ss1, sq1)
                    nc.scalar.activation(x_sh[:, sl], ps1[:], ACTF.Copy)
                stg = do_allreduce(2 + 3 * i, [slot_reduce(ss1, NCH_SH),
                                               slot_reduce(sq1, NCH_SH)])
                sc1, sh1 = finalize_bn(gc_sb[f"gm1{i}"], gc_sb[f"btm1{i}"],
                                       stg[:, 0:1], stg[:, 1:2])

                # y2 = relu(bn(m1)) * mask ; m2
                ss2 = stp.tile([F, NCH_SH], dt.float32, tag="ss2")
                sq2 = stp.tile([F, NCH_SH], dt.float32, tag="sq2")
                for c in range(NCH_SH):
                    sl = slice(c * CHUNK, (c + 1) * CHUNK)
                    y2 = tp.tile([F, CHUNK], dt.float16, tag="y2")
                    nc.scalar.activation(y2[:], x_sh[:, sl], ACTF.Relu,
                                         bias=sh1[:], scale=sc1[:])
                    nc.vector.tensor_mul(y2[:], y2[:], mask_sb[:, sl])
                    ps2 = ps_mm.tile([F, CHUNK], dt.float32, tag="mm")
                    nc.tensor.matmul(ps2[:], lhsT=W2_sb[i][:], rhs=y2[:],
                                     start=True, stop=True)
                    stat_pass(ps2[:], c, ss2, sq2)
                    nc.scalar.activation(x_sh[:, sl], ps2[:], ACTF.Copy)
                stg = do_allreduce(3 + 3 * i, [slot_reduce(ss2, NCH_SH),
                                               slot_reduce(sq2, NCH_SH)])
                sc2, sh2 = finalize_bn(gc_sb[f"gm2{i}"], gc_sb[f"btm2{i}"],
                                       stg[:, 0:1], stg[:, 1:2])

                # x_next = relu(bn(m2)) * mask (+ fp32 out on last layer)
                for c in range(NCH_SH):
                    sl = slice(c * CHUNK, (c + 1) * CHUNK)
                    if i == L - 1:
                        of = tp.tile([F, CHUNK], dt.float32, tag="of")
                        nc.scalar.activation(of[:], x_sh[:, sl], ACTF.Relu,
                                             bias=sh2[:], scale=sc2[:])
                        nc.sync.dma_start(out.ap()[:, sl], of[:])
                    else:
                        nc.scalar.activation(x_sh[:, sl], x_sh[:, sl],
                                             ACTF.Relu, bias=sh2[:],
                                             scale=sc2[:])
                        nc.vector.tensor_mul(x_sh[:, sl], x_sh[:, sl],
                                             mask_sb[:, sl])

    nc.compile()
    return nc


def kernel(**inputs):
    import sys
    for p in ("/opt/trn_rl_repo",):
        if p not in sys.path:
            sys.path.insert(0, p)
    from concourse import bass_utils

    meta = _prep(inputs["node_attr"], inputs["edge_index"], inputs["edge_attr"])

    nc = _build(meta)

    def col(v):
        return np.ascontiguousarray(v.astype(np.float32).reshape(F, 1))

    base = dict(
        W0=inputs["W0"].astype(BF16),
        g0=col(inputs["g0"]), bt0=col(inputs["bt0"]),
    )
    for i in range(L):
        base[f"Wn{i}"] = inputs["Wnode"][i].astype(BF16)
        base[f"Wb{i}"] = inputs["Wnb"][i].astype(BF16)
        base[f"W1{i}"] = inputs["Wm1"][i].astype(BF16)
        base[f"W2{i}"] = inputs["Wm2"][i].astype(BF16)
        base[f"Wec{i}"] = np.ascontiguousarray(
            np.stack([inputs["Wedge"][i][0], inputs["bedge"][i]]).astype(np.float32))
        base[f"Bnb{i}"] = np.ascontiguousarray(
            inputs["bnb"][i].astype(np.float32).reshape(1, F))
        for nm in ("gn", "btn", "ge", "bte", "gnb", "btnb",
                   "gm1", "btm1", "gm2", "btm2"):
            base[f"{nm}{i}"] = col(inputs[nm][i])

    in_maps = []
    for r in range(NCORES):
        m = dict(base)
        m["naT"] = meta["naT_r"][r]
        m["gidx"] = meta["gidx"][r]
        m["ohc"] = meta["ohc"][r]
        m["ohr"] = meta["ohr"][r]
        m["eav"] = meta["eav_t"][r]
        m["cot"] = meta["cot_r"][r]
        m["cin"] = meta["cin_r"][r]
        m["maskb"] = meta["mask_r"][r]
        in_maps.append(m)

    res = bass_utils.run_bass_kernel_spmd(
        nc, in_maps, core_ids=list(range(NCORES)))
    xT = np.concatenate([res.results[r]["out"] for r in range(NCORES)], axis=1)
    return np.ascontiguousarray(xT.T[:NREAL]).astype(np.float32)


if __name__ == "__main__":
    pass


# revision 28
# speedup vs baseline: 1.0532x; 1.0532x over previous
"""GNN message-passing kernel for Trainium2 (Bass/Tile), 8-core SPMD.

Strategy (v2 — sharded dense):
- Node tensors live TRANSPOSED on chip: xT [128 features (partitions), nodes].
- Nodes padded 40000 -> 40960 (320 blocks of 128); 8 cores; core r owns
  node shard [r*5120, (r+1)*5120) for BOTH the dense pipeline and the
  edge aggregation (edges assigned to the core owning their target col).
- Dense (x@W, BN, relu) is sharded: each core processes only its 5120
  nodes; BN batch stats are partial sums AllReduce'd as tiny [128,K]
  vectors (pad columns are kept exactly zero via a mask so masked sums
  equal sums over real nodes).
- Neighbor aggregation: h = x@Wnb computed for own shard in node-major
  PE blocks, written to DRAM and AllGather'd into a full h-table
  (node rows permuted as r*5120 + (n%5120%128)*40 + (n%5120//128) so the
  h-pass store is one contiguous DMA); per-edge rows fetched with
  dma_gather (int16 indices, two halves of the permuted node space) and
  scatter-added with PE one-hot matmuls (fp8 one-hot streamed from DRAM,
  built once on host from edge_index).
- Edge branch: ea = segment_sum(edge_attr@Wedge + bedge, row) factored as
  Wedge (x) s + bedge (x) c_out with s = segment_sum(edge_attr, row)
  computed on device by a row-sorted one-hot matmul pass (local to the
  shard; no collective needed); degree counts from host bincount.
  Biases followed by BN cancel inside BN and are dropped; bnb enters via
  c_in (x) bnb added to aggr before its BN.
"""
import numpy as np
import ml_dtypes

F = 128
L = 3
EPS = 1e-5
NREAL = 40000
NE = 640000
NCORES = 8
NP_ = 40960
SHARD = NP_ // NCORES       # 5120
HALF = NP_ // 2             # 20480
NBLK = SHARD // F           # 40 blocks per core
CHUNK = 512                 # dense free-dim chunk
NCH_SH = SHARD // CHUNK     # 10 dense chunks per core

BF16 = np.float16
FP8 = ml_dtypes.float8_e4m3


def _ceil(a, b):
    return -(-a // b)


def _wrap_idx16(vals):
    """int16 gather-index layout: value j at [j%16, j//16], tiled to 128 parts."""
    n = vals.shape[0]
    a = vals.reshape(n // 16, 16).T.astype(np.int16)   # [16, n/16]
    return np.tile(a, (8, 1))                          # [128, n/16]


def _prep(node_attr, edge_index, edge_attr):
    """Host-side index preprocessing -> per-core arrays + metadata."""
    row = edge_index[0].astype(np.int64)
    col = edge_index[1].astype(np.int64)
    ea = edge_attr[:, 0].astype(np.float32)

    # Source nodes are split into two "sub-shard halves" by block index within
    # the owning core's shard: A = blocks 0..19, B = blocks 20..39.  Each half
    # is AllGather'd separately (A first) so gathers on half A can start while
    # half B is still in flight.  Permuted h-table row index within a half:
    # rowX(n) = (n//SHARD)*(SHARD//2) + (n%SHARD%F)*(NBLK//2) + (b%(NBLK//2)),
    # so the h-pass writes each half with one contiguous [128, SHARD//2] DMA.
    rl = row % SHARD
    blk_src = rl // F
    half = (blk_src >= NBLK // 2).astype(np.int64)
    perm_in_half = (row // SHARD) * (SHARD // 2) + (rl % F) * (NBLK // 2) \
        + (blk_src - half * (NBLK // 2))

    shard = col // SHARD
    blk = (col % SHARD) // F
    tloc = col % F

    # --- col pass (neighbor aggregation) ---
    cnt = np.zeros((NCORES, 2, NBLK), np.int64)
    np.add.at(cnt, (shard, half, blk), 1)
    sseg = _ceil(max(int(cnt.max()), 1), F) * F
    cpb = sseg // F                   # chunks per (half, block)
    nch_h = NBLK * cpb                # chunks per half
    nstream_h = NBLK * sseg           # slots per half
    order = np.lexsort((blk, half, shard))
    so_shard, so_half, so_blk = shard[order], half[order], blk[order]
    grp = ((so_shard * 2 + so_half) * NBLK + so_blk)
    grp_start = np.zeros(NCORES * 2 * NBLK + 1, np.int64)
    np.add.at(grp_start, grp + 1, 1)
    grp_start = np.cumsum(grp_start)
    within = np.arange(NE) - grp_start[grp]
    slot = (so_half * NBLK + so_blk) * sseg + within   # slot in core stream

    gsrc = np.zeros((NCORES, 2 * nstream_h), np.int16)
    gsrc[so_shard, slot] = perm_in_half[order].astype(np.int16)
    oh_col = np.zeros((NCORES, 2 * nstream_h, F), np.uint8)
    oh_col[so_shard, slot, tloc[order]] = 1

    gidx = np.stack([_wrap_idx16(gsrc[r]) for r in range(NCORES)])  # [8,128,S/16]
    nch = 2 * nch_h
    g2 = 2 * cpb                      # one-hot chunks per DMA group
    # one-hot: [ngrp, 128 part, g2 chunk, 128] fp8
    ohc = oh_col.reshape(NCORES, nch // g2, g2, F, F).transpose(0, 1, 3, 2, 4)
    ohc = np.ascontiguousarray(ohc).astype(FP8)

    # --- row pass (edge branch s = segment_sum(edge_attr, row)) ---
    rshard = row // SHARD
    rblk = (row % SHARD) // F
    rloc = row % F
    rcnt = np.zeros((NCORES, NBLK), np.int64)
    np.add.at(rcnt, (rshard, rblk), 1)
    rseg = _ceil(max(int(rcnt.max()), 1), F) * F
    cpb2 = rseg // F
    nch2 = NBLK * cpb2
    rorder = np.lexsort((rblk, rshard))
    ro_shard, ro_blk = rshard[rorder], rblk[rorder]
    rgrp = ro_shard * NBLK + ro_blk
    rgs = np.zeros(NCORES * NBLK + 1, np.int64)
    np.add.at(rgs, rgrp + 1, 1)
    rgs = np.cumsum(rgs)
    rwithin = np.arange(NE) - rgs[rgrp]
    rslot = ro_blk * rseg + rwithin

    eav = np.zeros((NCORES, NBLK * rseg), np.float32)
    eav[ro_shard, rslot] = ea[rorder]
    oh_row = np.zeros((NCORES, NBLK * rseg, F), np.uint8)
    oh_row[ro_shard, rslot, rloc[rorder]] = 1
    ohr = oh_row.reshape(NCORES, nch2 // 8, 8, F, F).transpose(0, 1, 3, 2, 4)
    ohr = np.ascontiguousarray(ohr).astype(FP8)
    eav_t = np.ascontiguousarray(
        eav.reshape(NCORES, nch2, F).transpose(0, 2, 1)).astype(BF16)

    # degree counts (pure edge_index metadata), per-core slices
    c_out = np.bincount(row, minlength=NP_).astype(np.float32)
    c_in = np.bincount(col, minlength=NP_).astype(np.float32)
    cot_r = np.ascontiguousarray(c_out.reshape(NCORES, 1, SHARD)).astype(BF16)
    cin_r = np.ascontiguousarray(c_in.reshape(NCORES, 1, SHARD)).astype(BF16)

    naT = np.zeros((2, NP_), np.float32)
    naT[:, :NREAL] = node_attr.T
    naT_r = np.ascontiguousarray(
        naT.reshape(2, NCORES, SHARD).transpose(1, 0, 2)).astype(BF16)

    maskv = np.zeros(NP_, np.float32)
    maskv[:NREAL] = 1.0
    mask_r = np.ascontiguousarray(np.broadcast_to(
        maskv.reshape(NCORES, 1, SHARD), (NCORES, F, SHARD))
        .transpose(0, 1, 2)).astype(BF16)
    mask_r = np.ascontiguousarray(mask_r)

    return dict(sseg=sseg, cpb=cpb, nch=nch, rseg=rseg, cpb2=cpb2, nch2=nch2,
                gidx=gidx, ohc=ohc, ohr=ohr, eav_t=eav_t,
                cot_r=cot_r, cin_r=cin_r, naT_r=naT_r, mask_r=mask_r)


def _build(meta):
    """Build the Bass program. Returns nc."""
    import concourse.bass as bass
    import concourse.tile as tile
    from concourse import bacc, mybir

    cpb, nch, cpb2, nch2 = meta["cpb"], meta["nch"], meta["cpb2"], meta["nch2"]
    nch_h = nch // 2
    nstream_h = nch_h * F
    g2 = 2 * cpb                    # one-hot chunks per DMA group
    GCALL = 4 * cpb * F             # idxs per gather call -> 10 calls per half
    ncalls_h = nstream_h // GCALL
    dt = mybir.dt
    AX = mybir.AxisListType.X
    OP = mybir.AluOpType
    ACTF = mybir.ActivationFunctionType

    nc = bacc.Bacc("TRN2", target_bir_lowering=False, debug=False,
                   num_devices=NCORES, num_swdge_queues=4)

    # ---- DRAM tensors ----
    def din(name, shape, d):
        return nc.dram_tensor(name, shape, d, kind="ExternalInput")

    naT = din("naT", [2, SHARD], dt.float16)
    gidx = din("gidx", [128, 2 * nstream_h // 16], dt.int16)
    ohc = din("ohc", [nch // g2, 128, g2, F], dt.float8e4)
    ohr = din("ohr", [nch2 // 8, 128, 8, F], dt.float8e4)
    eav = din("eav", [128, nch2], dt.float16)
    cot = din("cot", [1, SHARD], dt.float16)
    cin = din("cin", [1, SHARD], dt.float16)
    maskb = din("maskb", [F, SHARD], dt.float16)
    W0 = din("W0", [2, F], dt.float16)
    Wn = [din(f"Wn{i}", [F, F], dt.float16) for i in range(L)]
    Wb = [din(f"Wb{i}", [F, F], dt.float16) for i in range(L)]
    W1 = [din(f"W1{i}", [F, F], dt.float16) for i in range(L)]
    W2 = [din(f"W2{i}", [F, F], dt.float16) for i in range(L)]
    Wec = [din(f"Wec{i}", [2, F], dt.float16) for i in range(L)]
    Bnb = [din(f"Bnb{i}", [1, F], dt.float16) for i in range(L)]
    gcol = {}
    for nm in ("g0", "bt0"):
        gcol[nm] = din(nm, [F, 1], dt.float32)
    for i in range(L):
        for nm in ("gn", "btn", "ge", "bte", "gnb", "btnb",
                   "gm1", "btm1", "gm2", "btm2"):
            gcol[f"{nm}{i}"] = din(f"{nm}{i}", [F, 1], dt.float32)

    out = nc.dram_tensor("out", [F, SHARD], dt.float32, kind="ExternalOutput")

    with tile.TileContext(nc) as tc:
        import contextlib
        ctx = contextlib.ExitStack()
        with ctx:
            sb = ctx.enter_context(tc.tile_pool(name="sb", bufs=1))
            wpool = ctx.enter_context(tc.tile_pool(name="wp", bufs=1))
            tp = ctx.enter_context(tc.tile_pool(name="tp", bufs=2))
            gp = ctx.enter_context(tc.tile_pool(name="gp", bufs=4))
            ohp = ctx.enter_context(tc.tile_pool(name="ohp", bufs=3))
            ohrp = ctx.enter_context(tc.tile_pool(name="ohrp", bufs=2))
            hst = ctx.enter_context(tc.tile_pool(name="hst", bufs=1))
            stp = ctx.enter_context(tc.tile_pool(name="stp", bufs=1))
            afp = ctx.enter_context(tc.tile_pool(name="afp", bufs=6))
            ps_mm = ctx.enter_context(tc.tile_pool(name="psmm", bufs=2, space="PSUM"))
            ps_sc = ctx.enter_context(tc.tile_pool(name="pssc", bufs=3, space="PSUM"))
            ps_ou = ctx.enter_context(tc.tile_pool(name="psou", bufs=2, space="PSUM"))
            ps_sp = ctx.enter_context(tc.tile_pool(name="pssp", bufs=1, space="PSUM"))
            dram = ctx.enter_context(tc.tile_pool(name="dram", bufs=1, space="DRAM"))

            # persistent SBUF
            x_sh = sb.tile([F, SHARD], dt.float16)
            xn_sh = sb.tile([F, SHARD], dt.float16)
            ea_sh = sb.tile([F, SHARD], dt.float16)
            aggr_sh = sb.tile([F, SHARD], dt.float32)
            mask_sb = sb.tile([F, SHARD], dt.float16)
            nc.sync.dma_start(mask_sb[:], maskb.ap())
            scs = sb.tile([2, SHARD], dt.float16)   # [s ; c_out]
            nc.sync.dma_start(scs[1:2, :], cot.ap())
            cii = sb.tile([1, SHARD], dt.float16)
            nc.sync.dma_start(cii[:], cin.ap())
            gidx_sb = sb.tile([128, 2 * nstream_h // 16], dt.int16)
            nc.sync.dma_start(gidx_sb[:], gidx.ap())
            eav_sb = sb.tile([128, nch2], dt.float16)
            nc.sync.dma_start(eav_sb[:], eav.ap())

            # weights resident
            W0_sb = wpool.tile([2, F], dt.float16)
            nc.sync.dma_start(W0_sb[:], W0.ap())
            Wn_sb, Wb_sb, W1_sb, W2_sb, Wec_sb, Bnb_sb = [], [], [], [], [], []
            for i in range(L):
                for lst, t_ in ((Wn_sb, Wn[i]), (Wb_sb, Wb[i]),
                                (W1_sb, W1[i]), (W2_sb, W2[i])):
                    w = wpool.tile([F, F], dt.float16, tag=f"w{len(lst)}_{t_.name}")
                    nc.sync.dma_start(w[:], t_.ap())
                    lst.append(w)
                w = wpool.tile([2, F], dt.float16, tag=f"wec{i}")
                nc.sync.dma_start(w[:], Wec[i].ap())
                Wec_sb.append(w)
                w = wpool.tile([1, F], dt.float16, tag=f"bnb{i}")
                nc.sync.dma_start(w[:], Bnb[i].ap())
                Bnb_sb.append(w)
            gc_sb = {}
            for nm, t_ in gcol.items():
                w = wpool.tile([F, 1], dt.float32, tag=f"p{nm}")
                nc.sync.dma_start(w[:], t_.ap())
                gc_sb[nm] = w

            # DRAM scratch: per layer, two half-shard h tables (A = source
            # blocks 0..19, B = 20..39), AllGather'd separately
            agh_ins = [[dram.tile([128, SHARD // 2], dt.float16,
                                  tag=f"ahi{i}{h}", name=f"ahi{i}{h}")
                        for h in range(2)] for i in range(L)]
            agh_outs = [[dram.tile([HALF, F], dt.float16, addr_space="Shared",
                                   tag=f"aho{i}{h}", name=f"aho{i}{h}")
                         for h in range(2)] for i in range(L)]
            st_ins, st_outs = [], []
            for k, w_ in enumerate([2] + [6, 2, 2] * L):
                st_ins.append(dram.tile([F, w_], dt.float32, tag=f"sti{k}",
                                        name=f"sti{k}"))
                st_outs.append(dram.tile([F, w_], dt.float32,
                                         tag=f"sto{k}", name=f"sto{k}"))

            # ---------- helpers ----------
            def finalize_bn(g, bt, ssum, ssq):
                """column affine from global sums: (scale, shift) [128,1] f32"""
                mean = afp.tile([F, 1], dt.float32)
                nc.vector.tensor_scalar_mul(mean[:], ssum, 1.0 / NREAL)
                m2t = afp.tile([F, 1], dt.float32)
                nc.scalar.activation(m2t[:], mean[:], ACTF.Square)
                var = afp.tile([F, 1], dt.float32)
                nc.vector.scalar_tensor_tensor(
                    out=var[:], in0=ssq, scalar=1.0 / NREAL, in1=m2t[:],
                    op0=OP.mult, op1=OP.subtract)
                nc.vector.tensor_scalar_add(var[:], var[:], EPS)
                lnv = afp.tile([F, 1], dt.float32)
                nc.scalar.activation(lnv[:], var[:], ACTF.Ln)
                isig = afp.tile([F, 1], dt.float32)
                nc.scalar.activation(isig[:], lnv[:], ACTF.Exp, scale=-0.5)
                scale = afp.tile([F, 1], dt.float32)
                nc.vector.tensor_mul(scale[:], g[:], isig[:])
                nscale = afp.tile([F, 1], dt.float32)
                nc.vector.tensor_scalar_mul(nscale[:], scale[:], -1.0)
                shift = afp.tile([F, 1], dt.float32)
                nc.vector.scalar_tensor_tensor(
                    out=shift[:], in0=mean[:], scalar=nscale[:], in1=bt[:],
                    op0=OP.mult, op1=OP.add)
                return scale, shift

            def stat_pass(src, c, ssum_sl, ssq_sl):
                """accumulate sum / sumsq of one [F, CHUNK] chunk into slot c"""
                nc.vector.tensor_reduce(ssum_sl[:, c:c + 1], src, AX, OP.add)
                tr = tp.tile([F, CHUNK], dt.float32, tag="tr")
                nc.scalar.activation(tr[:], src, ACTF.Square,
                                     accum_out=ssq_sl[:, c:c + 1])

            def slot_reduce(slots, n):
                r = afp.tile([F, 1], dt.float32)
                nc.vector.tensor_reduce(r[:], slots[:, :n], AX, OP.add)
                return r

            def do_allreduce(k, cols):
                """cols: list of [F,1] tiles -> AllReduce -> [F, len] tile"""
                w_ = len(cols)
                stc = stp.tile([F, 8], dt.float32, tag=f"stc{k}")
                for j, cl in enumerate(cols):
                    nc.vector.tensor_copy(stc[:, j:j + 1], cl[:])
                nc.sync.dma_start(st_ins[k][:, :], stc[:, :w_])
                nc.gpsimd.collective_compute(
                    "AllReduce", OP.add, replica_groups=[list(range(NCORES))],
                    ins=[st_ins[k].opt()], outs=[st_outs[k].opt()])
                stg = stp.tile([F, 8], dt.float32, tag=f"stg{k}")
                nc.sync.dma_start(stg[:, :w_], st_outs[k][:, :])
                return stg

            # ---------- layer 0: x0 = relu(bn0(naT @ W0)) * mask ----------
            ss0 = stp.tile([F, NCH_SH], dt.float32, tag="ss0")
            sq0 = stp.tile([F, NCH_SH], dt.float32, tag="sq0")
            for c in range(NCH_SH):
                sl = slice(c * CHUNK, (c + 1) * CHUNK)
                nat = tp.tile([2, CHUNK], dt.float16, tag="nat")
                nc.sync.dma_start(nat[:], naT.ap()[:, sl])
                ps = ps_mm.tile([F, CHUNK], dt.float32, tag="mm")
                nc.tensor.matmul(ps[:], lhsT=W0_sb[:], rhs=nat[:],
                                 start=True, stop=True)
                stat_pass(ps[:], c, ss0, sq0)
                nc.scalar.activation(x_sh[:, sl], ps[:], ACTF.Copy)
            stg = do_allreduce(0, [slot_reduce(ss0, NCH_SH),
                                   slot_reduce(sq0, NCH_SH)])
            sc0, sh0 = finalize_bn(gc_sb["g0"], gc_sb["bt0"],
                                   stg[:, 0:1], stg[:, 1:2])
            for c in range(NCH_SH):
                sl = slice(c * CHUNK, (c + 1) * CHUNK)
                nc.scalar.activation(x_sh[:, sl], x_sh[:, sl], ACTF.Relu,
                                     bias=sh0[:], scale=sc0[:])
                nc.vector.tensor_mul(x_sh[:, sl], x_sh[:, sl], mask_sb[:, sl])

            # ---------- s pass: s = segment_sum(edge_attr, row), local ----------
            for b in range(NBLK):
                pss = ps_sp.tile([1, F], dt.float32, tag="sp")
                for k in range(cpb2):
                    ci = b * cpb2 + k
                    g8 = ci // 8
                    if ci % 8 == 0:
                        ohrt = ohrp.tile([128, 8, F], dt.float8e4, tag="ohr")
                        nc.sync.dma_start(ohrt[:], ohr.ap()[g8])
                    nc.tensor.matmul(pss[:], lhsT=eav_sb[:, ci:ci + 1],
                                     rhs=ohrt[:, ci % 8, :],
                                     start=(k == 0), stop=(k == cpb2 - 1))
                nc.vector.tensor_copy(scs[0:1, b * F:(b + 1) * F], pss[:])

            # ---------- layers ----------
            gather_seq = [0]
            for i in range(L):
                # h-pass: node-major h = x @ Wb[i], one half-shard at a time;
                # AllGather A fires while B is still being computed
                for hh in range(2):
                    hs = hst.tile([128, SHARD // 2], dt.float16, tag=f"hs{hh}")
                    for bb in range(NBLK // 2):
                        b = hh * (NBLK // 2) + bb
                        ps = ps_sc.tile([F, F], dt.float32, tag="sc")
                        nc.tensor.matmul(ps[:], lhsT=x_sh[:, b * F:(b + 1) * F],
                                         rhs=Wb_sb[i][:], start=True, stop=True)
                        nc.vector.tensor_copy(hs[:, bb * F:(bb + 1) * F], ps[:])
                    nc.sync.dma_start(agh_ins[i][hh][:, :], hs[:])
                    nc.gpsimd.collective_compute(
                        "AllGather", OP.bypass,
                        replica_groups=[list(range(NCORES))],
                        ins=[agh_ins[i][hh].opt()], outs=[agh_outs[i][hh].opt()])

                # xn pass: xn = x @ Wn[i] (stored) + stats
                ssn = stp.tile([F, NCH_SH], dt.float32, tag="ssn")
                sqn = stp.tile([F, NCH_SH], dt.float32, tag="sqn")
                sse = stp.tile([F, NCH_SH], dt.float32, tag="sse")
                sqe = stp.tile([F, NCH_SH], dt.float32, tag="sqe")
                for c in range(NCH_SH):
                    sl = slice(c * CHUNK, (c + 1) * CHUNK)
                    ps = ps_mm.tile([F, CHUNK], dt.float32, tag="mm")
                    nc.tensor.matmul(ps[:], lhsT=Wn_sb[i][:], rhs=x_sh[:, sl],
                                     start=True, stop=True)
                    stat_pass(ps[:], c, ssn, sqn)
                    nc.vector.tensor_copy(xn_sh[:, sl], ps[:])
                    # ea chunk: rank-2 [Wedge;bedge] x [s;c_out]
                    pse = ps_ou.tile([F, CHUNK], dt.float32, tag="pse")
                    nc.tensor.matmul(pse[:], lhsT=Wec_sb[i][:], rhs=scs[:, sl],
                                     start=True, stop=True)
                    stat_pass(pse[:], c, sse, sqe)
                    nc.scalar.activation(ea_sh[:, sl], pse[:], ACTF.Copy)

                # scatter: per half, gather calls + one-hot matmuls -> aggr
                ssa = stp.tile([F, NBLK], dt.float32, tag="ssa")
                sqa = stp.tile([F, NBLK], dt.float32, tag="sqa")
                for h in range(2):
                    base = h * nstream_h
                    for call in range(ncalls_h):
                        gt = gp.tile([128, 4 * cpb, F], dt.float16, tag="g")
                        j0 = base + call * GCALL
                        # queue from a global counter: tile rotates DMASW sem
                        # lanes mod 8 per pool-DMA inst; lane L must always
                        # pair with queue L%4
                        nc.gpsimd.dma_gather(
                            out_ap=gt[:],
                            in_ap=agh_outs[i][h][:, :],
                            idxs_ap=gidx_sb[:, j0 // 16:(j0 + GCALL) // 16],
                            num_idxs=GCALL, num_idxs_reg=GCALL, elem_size=F,
                            queue_num=gather_seq[0] % 4,
                            single_packet=False)
                        gather_seq[0] += 1
                        for k8 in range(4 * cpb):
                            ci = call * 4 * cpb + k8    # chunk within half
                            gci = base // F + ci        # global chunk
                            if gci % g2 == 0:
                                oht = ohp.tile([128, g2, F], dt.float8e4,
                                               tag="oh")
                                nc.sync.dma_start(oht[:], ohc.ap()[gci // g2])
                            b = ci // cpb
                            k = ci % cpb
                            if k == 0:
                                psb = ps_sc.tile([F, F], dt.float32, tag="sc")
                            last = (k == cpb - 1)
                            nc.tensor.matmul(psb[:], lhsT=gt[:, k8, :],
                                             rhs=oht[:, gci % g2, :],
                                             start=(k == 0),
                                             stop=(last and h == 0))
                            if last:
                                dst = aggr_sh[:, b * F:(b + 1) * F]
                                if h == 0:
                                    nc.scalar.activation(dst, psb[:], ACTF.Copy)
                                else:
                                    # fold bnb (x) c_in into the same psum
                                    nc.tensor.matmul(
                                        psb[:], lhsT=Bnb_sb[i][:],
                                        rhs=cii[:, b * F:(b + 1) * F],
                                        start=False, stop=True)
                                    # aggr += psb ; stats
                                    nc.vector.scalar_tensor_tensor(
                                        out=dst, in0=psb[:], scalar=1.0,
                                        in1=dst, op0=OP.mult, op1=OP.add)
                                    nc.vector.tensor_reduce(
                                        ssa[:, b:b + 1], dst, AX, OP.add)
                                    tr8 = tp.tile([F, F], dt.float32, tag="tr8")
                                    nc.scalar.activation(
                                        tr8[:], dst, ACTF.Square,
                                        accum_out=sqa[:, b:b + 1])

                # AR#1: xn / ea / aggr stats
                stg = do_allreduce(1 + 3 * i, [
                    slot_reduce(ssn, NCH_SH), slot_reduce(sqn, NCH_SH),
                    slot_reduce(sse, NCH_SH), slot_reduce(sqe, NCH_SH),
                    slot_reduce(ssa, NBLK), slot_reduce(sqa, NBLK)])
                sc_n, sh_n = finalize_bn(gc_sb[f"gn{i}"], gc_sb[f"btn{i}"],
                                         stg[:, 0:1], stg[:, 1:2])
                sc_e, sh_e = finalize_bn(gc_sb[f"ge{i}"], gc_sb[f"bte{i}"],
                                         stg[:, 2:3], stg[:, 3:4])
                sc_a, sh_a = finalize_bn(gc_sb[f"gnb{i}"], gc_sb[f"btnb{i}"],
                                         stg[:, 4:5], stg[:, 5:6])
                shsum = afp.tile([F, 1], dt.float32)
                nc.vector.scalar_tensor_tensor(
                    out=shsum[:], in0=sh_n[:], scalar=1.0, in1=sh_e[:],
                    op0=OP.mult, op1=OP.add)
                nc.vector.scalar_tensor_tensor(
                    out=shsum[:], in0=shsum[:], scalar=1.0, in1=sh_a[:],
                    op0=OP.mult, op1=OP.add)

                # y1 = relu(sc_n*xn + sc_a*aggr + sc_e*ea + shsum) * mask ; m1
                ss1 = stp.tile([F, NCH_SH], dt.float32, tag="ss1")
                sq1 = stp.tile([F, NCH_SH], dt.float32, tag="sq1")
                for c in range(NCH_SH):
                    sl = slice(c * CHUNK, (c + 1) * CHUNK)
                    u1 = tp.tile([F, CHUNK], dt.float32, tag="u1")
                    nc.scalar.activation(u1[:], xn_sh[:, sl], ACTF.Copy,
                                         scale=sc_n[:])
                    u2 = tp.tile([F, CHUNK], dt.float32, tag="u2")
                    nc.vector.scalar_tensor_tensor(
                        out=u2[:], in0=aggr_sh[:, sl], scalar=sc_a[:],
                        in1=u1[:], op0=OP.mult, op1=OP.add)
                    nc.vector.scalar_tensor_tensor(
                        out=u2[:], in0=ea_sh[:, sl], scalar=sc_e[:],
                        in1=u2[:], op0=OP.mult, op1=OP.add)
                    y1 = tp.tile([F, CHUNK], dt.float16, tag="y1")
                    nc.scalar.activation(y1[:], u2[:], ACTF.Relu,
                                         bias=shsum[:], scale=1.0)
                    nc.vector.tensor_mul(y1[:], y1[:], mask_sb[:, sl])
                    ps1 = ps_mm.tile([F, CHUNK], dt.float32, tag="mm")
                    nc.tensor.matmul(ps1[:], lhsT=W1_sb[i][:], rhs=y1[:],
                                     start=True, stop=True)
                    stat_pass(ps1[:], c, ss1, sq1)
                    nc.scalar.activation(x_sh[:, sl], ps1[:], ACTF.Copy)
                stg = do_allreduce(2 + 3 * i, [slot_reduce(ss1, NCH_SH),
                                               slot_reduce(sq1, NCH_SH)])
                sc1, sh1 = finalize_bn(gc_sb[f"gm1{i}"], gc_sb[f"btm1{i}"],
                                       stg[:, 0:1], stg[:, 1:2])

                # y2 = relu(bn(m1)) * mask ; m2
                ss2 = stp.tile([F, NCH_SH], dt.float32, tag="ss2")
                sq2 = stp.tile([F, NCH_SH], dt.float32, tag="sq2")
                for c in range(NCH_SH):
                    sl = slice(c * CHUNK, (c + 1) * CHUNK)
                    y2 = tp.tile([F, CHUNK], dt.float16, tag="y2")
                    nc.scalar.activation(y2[:], x_sh[:, sl], ACTF.Relu,
                                         bias=sh1[:], scale=sc1[:])
                    nc.vector.tensor_mul(y2[:], y2[:], mask_sb[:, sl])
                    ps2 = ps_mm.tile([F, CHUNK], dt.float32, tag="mm")
                    nc.tensor.matmul(ps2[:], lhsT=W2_sb[i][:], rhs=y2[:],
                                     start=True, stop=True)
                    stat_pass(ps2[:], c, ss2, sq2)
                    nc.scalar.activation(x_sh[:, sl], ps2[:], ACTF.Copy)
                stg = do_allreduce(3 + 3 * i, [slot_reduce(ss2, NCH_SH),
                                               slot_reduce(sq2, NCH_SH)])
                sc2, sh2 = finalize_bn(gc_sb[f"gm2{i}"], gc_sb[f"btm2{i}"],
                                       stg[:, 0:1], stg[:, 1:2])

                # x_next = relu(bn(m2)) * mask (+ fp32 out on last layer)
                for c in range(NCH_SH):
                    sl = slice(c * CHUNK, (c + 1) * CHUNK)
                    if i == L - 1:
                        of = tp.tile([F, CHUNK], dt.float32, tag="of")
                        nc.scalar.activation(of[:], x_sh[:, sl], ACTF.Relu,
                                             bias=sh2[:], scale=sc2[:])
                        nc.sync.dma_start(out.ap()[:, sl], of[:])
                    else:
                        nc.scalar.activation(x_sh[:, sl], x_sh[:, sl],
                                             ACTF.Relu, bias=sh2[:],
                                             scale=sc2[:])
                        nc.vector.tensor_mul(x_sh[:, sl], x_sh[:, sl],
                                             mask_sb[:, sl])

    nc.compile()
    return nc


def kernel(**inputs):
    import sys
    for p in ("/opt/trn_rl_repo",):
        if p not in sys.path:
            sys.path.insert(0, p)
    from concourse import bass_utils

    meta = _prep(inputs["node_attr"], inputs["edge_index"], inputs["edge_attr"])

    nc = _build(meta)

    def col(v):
        return np.ascontiguousarray(v.astype(np.float32).reshape(F, 1))

    base = dict(
        W0=inputs["W0"].astype(BF16),
        g0=col(inputs["g0"]), bt0=col(inputs["bt0"]),
    )
    for i in range(L):
        base[f"Wn{i}"] = inputs["Wnode"][i].astype(BF16)
        base[f"Wb{i}"] = inputs["Wnb"][i].astype(BF16)
        base[f"W1{i}"] = inputs["Wm1"][i].astype(BF16)
        base[f"W2{i}"] = inputs["Wm2"][i].astype(BF16)
        base[f"Wec{i}"] = np.ascontiguousarray(
            np.stack([inputs["Wedge"][i][0], inputs["bedge"][i]]).astype(BF16))
        base[f"Bnb{i}"] = np.ascontiguousarray(
            inputs["bnb"][i].astype(BF16).reshape(1, F))
        for nm in ("gn", "btn", "ge", "bte", "gnb", "btnb",
                   "gm1", "btm1", "gm2", "btm2"):
            base[f"{nm}{i}"] = col(inputs[nm][i])

    in_maps = []
    for r in range(NCORES):
        m = dict(base)
        m["naT"] = meta["naT_r"][r]
        m["gidx"] = meta["gidx"][r]
        m["ohc"] = meta["ohc"][r]
        m["ohr"] = meta["ohr"][r]
        m["eav"] = meta["eav_t"][r]
        m["cot"] = meta["cot_r"][r]
        m["cin"] = meta["cin_r"][r]
        m["maskb"] = meta["mask_r"][r]
        in_maps.append(m)

    res = bass_utils.run_bass_kernel_spmd(
        nc, in_maps, core_ids=list(range(NCORES)))
    xT = np.concatenate([res.results[r]["out"] for r in range(NCORES)], axis=1)
    return np.ascontiguousarray(xT.T[:NREAL]).astype(np.float32)


if __name__ == "__main__":
    pass


# revision 32
# speedup vs baseline: 1.0778x; 1.0233x over previous
"""GNN message-passing kernel for Trainium2 (Bass/Tile), 8-core SPMD.

Strategy (v2 — sharded dense):
- Node tensors live TRANSPOSED on chip: xT [128 features (partitions), nodes].
- Nodes padded 40000 -> 40960 (320 blocks of 128); 8 cores; core r owns
  node shard [r*5120, (r+1)*5120) for BOTH the dense pipeline and the
  edge aggregation (edges assigned to the core owning their target col).
- Dense (x@W, BN, relu) is sharded: each core processes only its 5120
  nodes; BN batch stats are partial sums AllReduce'd as tiny [128,K]
  vectors (pad columns are kept exactly zero via a mask so masked sums
  equal sums over real nodes).
- Neighbor aggregation: h = x@Wnb computed for own shard in node-major
  PE blocks, written to DRAM and AllGather'd into a full h-table
  (node rows permuted as r*5120 + (n%5120%128)*40 + (n%5120//128) so the
  h-pass store is one contiguous DMA); per-edge rows fetched with
  dma_gather (int16 indices, two halves of the permuted node space) and
  scatter-added with PE one-hot matmuls (fp8 one-hot streamed from DRAM,
  built once on host from edge_index).
- Edge branch: ea = segment_sum(edge_attr@Wedge + bedge, row) factored as
  Wedge (x) s + bedge (x) c_out with s = segment_sum(edge_attr, row)
  computed on device by a row-sorted one-hot matmul pass (local to the
  shard; no collective needed); degree counts from host bincount.
  Biases followed by BN cancel inside BN and are dropped; bnb enters via
  c_in (x) bnb added to aggr before its BN.
"""
import numpy as np
import ml_dtypes

F = 128
L = 3
EPS = 1e-5
NREAL = 40000
NE = 640000
NCORES = 8
NP_ = 40960
SHARD = NP_ // NCORES       # 5120
HALF = NP_ // 2             # 20480
NBLK = SHARD // F           # 40 blocks per core
CHUNK = 512                 # dense free-dim chunk
NCH_SH = SHARD // CHUNK     # 10 dense chunks per core

BF16 = np.float16
FP8 = ml_dtypes.float8_e4m3


def _ceil(a, b):
    return -(-a // b)


def _wrap_idx16(vals):
    """int16 gather-index layout: value j at [j%16, j//16], tiled to 128 parts."""
    n = vals.shape[0]
    a = vals.reshape(n // 16, 16).T.astype(np.int16)   # [16, n/16]
    return np.tile(a, (8, 1))                          # [128, n/16]


def _prep(node_attr, edge_index, edge_attr):
    """Host-side index preprocessing -> per-core arrays + metadata."""
    row = edge_index[0].astype(np.int64)
    col = edge_index[1].astype(np.int64)
    ea = edge_attr[:, 0].astype(np.float32)

    # Source nodes are split into two "sub-shard halves" by block index within
    # the owning core's shard: A = blocks 0..19, B = blocks 20..39.  Each half
    # is AllGather'd separately (A first) so gathers on half A can start while
    # half B is still in flight.  Permuted h-table row index within a half:
    # rowX(n) = (n//SHARD)*(SHARD//2) + (n%SHARD%F)*(NBLK//2) + (b%(NBLK//2)),
    # so the h-pass writes each half with one contiguous [128, SHARD//2] DMA.
    rl = row % SHARD
    blk_src = rl // F
    half = (blk_src >= NBLK // 2).astype(np.int64)
    perm_in_half = (row // SHARD) * (SHARD // 2) + (rl % F) * (NBLK // 2) \
        + (blk_src - half * (NBLK // 2))

    shard = col // SHARD
    blk = (col % SHARD) // F
    tloc = col % F

    # --- col pass (neighbor aggregation) ---
    # stream 0: "own" edges (source in the same shard as the target) read a
    # locally-written table (no collective dependency); streams 1/2 read the
    # AllGather'd sub-shard halves A/B.  Own-table row: hh*2560 + p*20 + bb.
    own = (row // SHARD) == shard
    strm = np.where(own, 0, 1 + half)
    perm_own = half * (SHARD // 2) + (rl % F) * (NBLK // 2) \
        + (blk_src - half * (NBLK // 2))
    gval = np.where(own, perm_own, perm_in_half)

    cnt_o = np.zeros((NCORES, NBLK), np.int64)
    np.add.at(cnt_o, (shard[own], blk[own]), 1)
    sseg_o = _ceil(max(int(cnt_o.max()), 1), F) * F
    cpb_o = sseg_o // F
    cnt_r = np.zeros((NCORES, 2, NBLK), np.int64)
    np.add.at(cnt_r, (shard[~own], half[~own], blk[~own]), 1)
    sseg = _ceil(max(int(cnt_r.max()), 1), F) * F
    cpb = sseg // F                   # chunks per (remote half, block)
    nch_h = NBLK * cpb                # chunks per remote half
    nstream_o = NBLK * sseg_o         # own-stream slots
    nstream_h = NBLK * sseg           # slots per remote half
    nslots = nstream_o + 2 * nstream_h
    order = np.lexsort((blk, strm, shard))
    so_shard, so_strm, so_blk = shard[order], strm[order], blk[order]
    grp = (so_shard * 3 + so_strm) * NBLK + so_blk
    grp_start = np.zeros(NCORES * 3 * NBLK + 1, np.int64)
    np.add.at(grp_start, grp + 1, 1)
    grp_start = np.cumsum(grp_start)
    within = np.arange(NE) - grp_start[grp]
    sbase = np.where(so_strm == 0, 0, nstream_o + (so_strm - 1) * nstream_h)
    sseg_of = np.where(so_strm == 0, sseg_o, sseg)
    slot = sbase + so_blk * sseg_of + within   # slot in core stream

    gsrc = np.zeros((NCORES, nslots), np.int16)
    gsrc[so_shard, slot] = gval[order].astype(np.int16)
    oh_col = np.zeros((NCORES, nslots, F), np.uint8)
    oh_col[so_shard, slot, tloc[order]] = 1

    gidx = np.stack([_wrap_idx16(gsrc[r]) for r in range(NCORES)])  # [8,128,S/16]
    nch_o = NBLK * cpb_o
    nch = nch_o + 2 * nch_h
    g2o = 2 * cpb_o                   # own one-hot chunks per DMA group
    g2 = 2 * cpb                      # remote one-hot chunks per DMA group
    oh_own = oh_col[:, :nstream_o].reshape(NCORES, nch_o // g2o, g2o, F, F)
    oh_own = np.ascontiguousarray(oh_own.transpose(0, 1, 3, 2, 4)).astype(FP8)
    oh_rem = oh_col[:, nstream_o:].reshape(NCORES, 2 * nch_h // g2, g2, F, F)
    oh_rem = np.ascontiguousarray(oh_rem.transpose(0, 1, 3, 2, 4)).astype(FP8)

    # --- row pass (edge branch s = segment_sum(edge_attr, row)) ---
    rshard = row // SHARD
    rblk = (row % SHARD) // F
    rloc = row % F
    rcnt = np.zeros((NCORES, NBLK), np.int64)
    np.add.at(rcnt, (rshard, rblk), 1)
    rseg = _ceil(max(int(rcnt.max()), 1), F) * F
    cpb2 = rseg // F
    nch2 = NBLK * cpb2
    rorder = np.lexsort((rblk, rshard))
    ro_shard, ro_blk = rshard[rorder], rblk[rorder]
    rgrp = ro_shard * NBLK + ro_blk
    rgs = np.zeros(NCORES * NBLK + 1, np.int64)
    np.add.at(rgs, rgrp + 1, 1)
    rgs = np.cumsum(rgs)
    rwithin = np.arange(NE) - rgs[rgrp]
    rslot = ro_blk * rseg + rwithin

    eav = np.zeros((NCORES, NBLK * rseg), np.float32)
    eav[ro_shard, rslot] = ea[rorder]
    oh_row = np.zeros((NCORES, NBLK * rseg, F), np.uint8)
    oh_row[ro_shard, rslot, rloc[rorder]] = 1
    ohr = oh_row.reshape(NCORES, nch2 // 8, 8, F, F).transpose(0, 1, 3, 2, 4)
    ohr = np.ascontiguousarray(ohr).astype(FP8)
    eav_t = np.ascontiguousarray(
        eav.reshape(NCORES, nch2, F).transpose(0, 2, 1)).astype(BF16)

    # degree counts (pure edge_index metadata), per-core slices
    c_out = np.bincount(row, minlength=NP_).astype(np.float32)
    c_in = np.bincount(col, minlength=NP_).astype(np.float32)
    cot_r = np.ascontiguousarray(c_out.reshape(NCORES, 1, SHARD)).astype(BF16)
    cin_r = np.ascontiguousarray(c_in.reshape(NCORES, 1, SHARD)).astype(BF16)

    naT = np.zeros((2, NP_), np.float32)
    naT[:, :NREAL] = node_attr.T
    naT_r = np.ascontiguousarray(
        naT.reshape(2, NCORES, SHARD).transpose(1, 0, 2)).astype(BF16)

    maskv = np.zeros(NP_, np.float32)
    maskv[:NREAL] = 1.0
    mask_r = np.ascontiguousarray(np.broadcast_to(
        maskv.reshape(NCORES, 1, SHARD), (NCORES, F, SHARD))
        .transpose(0, 1, 2)).astype(BF16)
    mask_r = np.ascontiguousarray(mask_r)

    return dict(cpb=cpb, cpb_o=cpb_o, cpb2=cpb2, nch2=nch2,
                gidx=gidx, ohc=oh_own, ohcr=oh_rem, ohr=ohr, eav_t=eav_t,
                cot_r=cot_r, cin_r=cin_r, naT_r=naT_r, mask_r=mask_r)


def _build(meta):
    """Build the Bass program. Returns nc."""
    import concourse.bass as bass
    import concourse.tile as tile
    from concourse import bacc, mybir

    cpb, cpb_o, cpb2, nch2 = (meta["cpb"], meta["cpb_o"], meta["cpb2"],
                              meta["nch2"])
    nch_h = NBLK * cpb
    nch_o = NBLK * cpb_o
    nstream_h = nch_h * F
    nstream_o = nch_o * F
    nslots = nstream_o + 2 * nstream_h
    g2 = 2 * cpb                    # remote one-hot chunks per DMA group
    g2o = 2 * cpb_o                 # own one-hot chunks per DMA group
    GCALL = 4 * cpb * F             # idxs per remote gather call (10 per half)
    GCALL_O = 4 * cpb_o * F         # idxs per own gather call (10 calls)
    ncalls_h = nstream_h // GCALL
    ncalls_o = nstream_o // GCALL_O
    dt = mybir.dt
    AX = mybir.AxisListType.X
    OP = mybir.AluOpType
    ACTF = mybir.ActivationFunctionType

    nc = bacc.Bacc("TRN2", target_bir_lowering=False, debug=False,
                   num_devices=NCORES, num_swdge_queues=4)

    # ---- DRAM tensors ----
    def din(name, shape, d):
        return nc.dram_tensor(name, shape, d, kind="ExternalInput")

    naT = din("naT", [2, SHARD], dt.float16)
    gidx = din("gidx", [128, nslots // 16], dt.int16)
    ohc = din("ohc", [nch_o // g2o, 128, g2o, F], dt.float8e4)
    ohcr = din("ohcr", [2 * nch_h // g2, 128, g2, F], dt.float8e4)
    ohr = din("ohr", [nch2 // 8, 128, 8, F], dt.float8e4)
    eav = din("eav", [128, nch2], dt.float16)
    cot = din("cot", [1, SHARD], dt.float16)
    cin = din("cin", [1, SHARD], dt.float16)
    maskb = din("maskb", [F, SHARD], dt.float16)
    W0 = din("W0", [2, F], dt.float16)
    Wn = [din(f"Wn{i}", [F, F], dt.float16) for i in range(L)]
    Wb = [din(f"Wb{i}", [F, F], dt.float16) for i in range(L)]
    W1 = [din(f"W1{i}", [F, F], dt.float16) for i in range(L)]
    W2 = [din(f"W2{i}", [F, F], dt.float16) for i in range(L)]
    Wec = [din(f"Wec{i}", [2, F], dt.float16) for i in range(L)]
    Bnb = [din(f"Bnb{i}", [1, F], dt.float16) for i in range(L)]
    gcol = {}
    for nm in ("g0", "bt0"):
        gcol[nm] = din(nm, [F, 1], dt.float32)
    for i in range(L):
        for nm in ("gn", "btn", "ge", "bte", "gnb", "btnb",
                   "gm1", "btm1", "gm2", "btm2"):
            gcol[f"{nm}{i}"] = din(f"{nm}{i}", [F, 1], dt.float32)

    out = nc.dram_tensor("out", [F, SHARD], dt.float32, kind="ExternalOutput")

    with tile.TileContext(nc) as tc:
        import contextlib
        ctx = contextlib.ExitStack()
        with ctx:
            sb = ctx.enter_context(tc.tile_pool(name="sb", bufs=1))
            wpool = ctx.enter_context(tc.tile_pool(name="wp", bufs=1))
            tp = ctx.enter_context(tc.tile_pool(name="tp", bufs=2))
            gp = ctx.enter_context(tc.tile_pool(name="gp", bufs=4))
            ohp = ctx.enter_context(tc.tile_pool(name="ohp", bufs=3))
            ohrp = ctx.enter_context(tc.tile_pool(name="ohrp", bufs=2))
            hst = ctx.enter_context(tc.tile_pool(name="hst", bufs=1))
            stp = ctx.enter_context(tc.tile_pool(name="stp", bufs=1))
            afp = ctx.enter_context(tc.tile_pool(name="afp", bufs=6))
            ps_mm = ctx.enter_context(tc.tile_pool(name="psmm", bufs=2, space="PSUM"))
            ps_sc = ctx.enter_context(tc.tile_pool(name="pssc", bufs=3, space="PSUM"))
            ps_ou = ctx.enter_context(tc.tile_pool(name="psou", bufs=2, space="PSUM"))
            ps_sp = ctx.enter_context(tc.tile_pool(name="pssp", bufs=1, space="PSUM"))
            dram = ctx.enter_context(tc.tile_pool(name="dram", bufs=1, space="DRAM"))

            # persistent SBUF.  Small weight/param loads are issued FIRST so
            # layer 0's matmuls don't queue behind the multi-MB index loads.
            W0_sb = wpool.tile([2, F], dt.float16)
            nc.sync.dma_start(W0_sb[:], W0.ap())
            x_sh = sb.tile([F, SHARD], dt.float16)
            xn_sh = sb.tile([F, SHARD], dt.float16)
            ea_sh = sb.tile([F, SHARD], dt.float16)
            aggr_sh = sb.tile([F, SHARD], dt.float32)
            mask_sb = sb.tile([F, SHARD], dt.float16)
            scs = sb.tile([2, SHARD], dt.float16)   # [s ; c_out]
            cii = sb.tile([1, SHARD], dt.float16)
            gidx_sb = sb.tile([128, nslots // 16], dt.int16)
            eav_sb = sb.tile([128, nch2], dt.float16)
            Wn_sb, Wb_sb, W1_sb, W2_sb, Wec_sb, Bnb_sb = [], [], [], [], [], []
            for i in range(L):
                for lst, t_ in ((Wn_sb, Wn[i]), (Wb_sb, Wb[i]),
                                (W1_sb, W1[i]), (W2_sb, W2[i])):
                    w = wpool.tile([F, F], dt.float16, tag=f"w{len(lst)}_{t_.name}")
                    nc.sync.dma_start(w[:], t_.ap())
                    lst.append(w)
                w = wpool.tile([2, F], dt.float16, tag=f"wec{i}")
                nc.sync.dma_start(w[:], Wec[i].ap())
                Wec_sb.append(w)
                w = wpool.tile([1, F], dt.float16, tag=f"bnb{i}")
                nc.sync.dma_start(w[:], Bnb[i].ap())
                Bnb_sb.append(w)
            gc_sb = {}
            for nm, t_ in gcol.items():
                w = wpool.tile([F, 1], dt.float32, tag=f"p{nm}")
                nc.sync.dma_start(w[:], t_.ap())
                gc_sb[nm] = w
            # bulk loads after the small ones
            nc.sync.dma_start(mask_sb[:], maskb.ap())
            nc.sync.dma_start(scs[1:2, :], cot.ap())
            nc.sync.dma_start(cii[:], cin.ap())
            nc.sync.dma_start(gidx_sb[:], gidx.ap())
            nc.sync.dma_start(eav_sb[:], eav.ap())

            # DRAM scratch: per layer, two half-shard h tables (A = source
            # blocks 0..19, B = 20..39), AllGather'd separately
            agh_ins = [[dram.tile([128, SHARD // 2], dt.float16,
                                  tag=f"ahi{i}{h}", name=f"ahi{i}{h}")
                        for h in range(2)] for i in range(L)]
            agh_outs = [[dram.tile([HALF, F], dt.float16, addr_space="Shared",
                                   tag=f"aho{i}{h}", name=f"aho{i}{h}")
                         for h in range(2)] for i in range(L)]
            htab_own = dram.tile([SHARD, F], dt.float16, tag="hto",
                                 name="hto")
            st_ins, st_outs = [], []
            for k, w_ in enumerate([2] + [6, 2, 2] * L):
                st_ins.append(dram.tile([F, w_], dt.float32, tag=f"sti{k}",
                                        name=f"sti{k}"))
                st_outs.append(dram.tile([F, w_], dt.float32,
                                         tag=f"sto{k}", name=f"sto{k}"))

            # ---------- helpers ----------
            def finalize_bn(g, bt, ssum, ssq):
                """column affine from global sums: (scale, shift) [128,1] f32"""
                mean = afp.tile([F, 1], dt.float32)
                nc.vector.tensor_scalar_mul(mean[:], ssum, 1.0 / NREAL)
                m2t = afp.tile([F, 1], dt.float32)
                nc.scalar.activation(m2t[:], mean[:], ACTF.Square)
                var = afp.tile([F, 1], dt.float32)
                nc.vector.scalar_tensor_tensor(
                    out=var[:], in0=ssq, scalar=1.0 / NREAL, in1=m2t[:],
                    op0=OP.mult, op1=OP.subtract)
                nc.vector.tensor_scalar_add(var[:], var[:], EPS)
                lnv = afp.tile([F, 1], dt.float32)
                nc.scalar.activation(lnv[:], var[:], ACTF.Ln)
                isig = afp.tile([F, 1], dt.float32)
                nc.scalar.activation(isig[:], lnv[:], ACTF.Exp, scale=-0.5)
                scale = afp.tile([F, 1], dt.float32)
                nc.vector.tensor_mul(scale[:], g[:], isig[:])
                nscale = afp.tile([F, 1], dt.float32)
                nc.vector.tensor_scalar_mul(nscale[:], scale[:], -1.0)
                shift = afp.tile([F, 1], dt.float32)
                nc.vector.scalar_tensor_tensor(
                    out=shift[:], in0=mean[:], scalar=nscale[:], in1=bt[:],
                    op0=OP.mult, op1=OP.add)
                return scale, shift

            def stat_pass(src, c, ssum_sl, ssq_sl):
                """accumulate sum / sumsq of one [F, CHUNK] chunk into slot c"""
                nc.vector.tensor_reduce(ssum_sl[:, c:c + 1], src, AX, OP.add)
                tr = tp.tile([F, CHUNK], dt.float32, tag="tr")
                nc.scalar.activation(tr[:], src, ACTF.Square,
                                     accum_out=ssq_sl[:, c:c + 1])

            def slot_reduce(slots, n):
                r = afp.tile([F, 1], dt.float32)
                nc.vector.tensor_reduce(r[:], slots[:, :n], AX, OP.add)
                return r

            def do_allreduce(k, cols):
                """cols: list of [F,1] tiles -> AllReduce -> [F, len] tile"""
                w_ = len(cols)
                stc = stp.tile([F, 8], dt.float32, tag=f"stc{k}")
                for j, cl in enumerate(cols):
                    nc.vector.tensor_copy(stc[:, j:j + 1], cl[:])
                nc.sync.dma_start(st_ins[k][:, :], stc[:, :w_])
                nc.gpsimd.collective_compute(
                    "AllReduce", OP.add, replica_groups=[list(range(NCORES))],
                    ins=[st_ins[k].opt()], outs=[st_outs[k].opt()])
                stg = stp.tile([F, 8], dt.float32, tag=f"stg{k}")
                nc.sync.dma_start(stg[:, :w_], st_outs[k][:, :])
                return stg

            # ---------- layer 0: x0 = relu(bn0(naT @ W0)) * mask ----------
            ss0 = stp.tile([F, NCH_SH], dt.float32, tag="ss0")
            sq0 = stp.tile([F, NCH_SH], dt.float32, tag="sq0")
            for c in range(NCH_SH):
                sl = slice(c * CHUNK, (c + 1) * CHUNK)
                nat = tp.tile([2, CHUNK], dt.float16, tag="nat")
                nc.sync.dma_start(nat[:], naT.ap()[:, sl])
                ps = ps_mm.tile([F, CHUNK], dt.float32, tag="mm")
                nc.tensor.matmul(ps[:], lhsT=W0_sb[:], rhs=nat[:],
                                 start=True, stop=True)
                stat_pass(ps[:], c, ss0, sq0)
                nc.scalar.activation(x_sh[:, sl], ps[:], ACTF.Copy)
            stg = do_allreduce(0, [slot_reduce(ss0, NCH_SH),
                                   slot_reduce(sq0, NCH_SH)])
            sc0, sh0 = finalize_bn(gc_sb["g0"], gc_sb["bt0"],
                                   stg[:, 0:1], stg[:, 1:2])
            for c in range(NCH_SH):
                sl = slice(c * CHUNK, (c + 1) * CHUNK)
                nc.scalar.activation(x_sh[:, sl], x_sh[:, sl], ACTF.Relu,
                                     bias=sh0[:], scale=sc0[:])
                nc.vector.tensor_mul(x_sh[:, sl], x_sh[:, sl], mask_sb[:, sl])

            # ---------- layers ----------
            gather_seq = [0]
            for i in range(L):
                # h-pass: node-major h = x @ Wb[i], one half-shard at a time;
                # AllGather A fires while B is still being computed
                for hh in range(2):
                    hs = hst.tile([128, SHARD // 2], dt.float16, tag=f"hs{hh}")
                    for bb in range(NBLK // 2):
                        b = hh * (NBLK // 2) + bb
                        ps = ps_sc.tile([F, F], dt.float32, tag="sc")
                        nc.tensor.matmul(ps[:], lhsT=x_sh[:, b * F:(b + 1) * F],
                                         rhs=Wb_sb[i][:], start=True, stop=True)
                        nc.vector.tensor_copy(hs[:, bb * F:(bb + 1) * F], ps[:])
                    nc.sync.dma_start(
                        htab_own.ap()[hh * (SHARD // 2):(hh + 1) * (SHARD // 2), :],
                        hs[:])
                    nc.sync.dma_start(agh_ins[i][hh][:, :], hs[:])
                    nc.gpsimd.collective_compute(
                        "AllGather", OP.bypass,
                        replica_groups=[list(range(NCORES))],
                        ins=[agh_ins[i][hh].opt()], outs=[agh_outs[i][hh].opt()])

                if i == 0:
                    # s pass: s = segment_sum(edge_attr, row), local.  Placed
                    # after layer 0's AllGather issue so its PE work overlaps
                    # the collective instead of delaying it.
                    for b in range(NBLK):
                        pss = ps_sp.tile([1, F], dt.float32, tag="sp")
                        for k in range(cpb2):
                            ci = b * cpb2 + k
                            if ci % 8 == 0:
                                ohrt = ohrp.tile([128, 8, F], dt.float8e4,
                                                 tag="ohr")
                                nc.sync.dma_start(ohrt[:], ohr.ap()[ci // 8])
                            nc.tensor.matmul(pss[:], lhsT=eav_sb[:, ci:ci + 1],
                                             rhs=ohrt[:, ci % 8, :],
                                             start=(k == 0),
                                             stop=(k == cpb2 - 1))
                        nc.vector.tensor_copy(scs[0:1, b * F:(b + 1) * F],
                                              pss[:])

                # xn pass: xn = x @ Wn[i] (stored) + stats
                ssn = stp.tile([F, NCH_SH], dt.float32, tag="ssn")
                sqn = stp.tile([F, NCH_SH], dt.float32, tag="sqn")
                sse = stp.tile([F, NCH_SH], dt.float32, tag="sse")
                sqe = stp.tile([F, NCH_SH], dt.float32, tag="sqe")
                for c in range(NCH_SH):
                    sl = slice(c * CHUNK, (c + 1) * CHUNK)
                    ps = ps_mm.tile([F, CHUNK], dt.float32, tag="mm")
                    nc.tensor.matmul(ps[:], lhsT=Wn_sb[i][:], rhs=x_sh[:, sl],
                                     start=True, stop=True)
                    stat_pass(ps[:], c, ssn, sqn)
                    nc.vector.tensor_copy(xn_sh[:, sl], ps[:])
                    # ea chunk: rank-2 [Wedge;bedge] x [s;c_out]
                    pse = ps_ou.tile([F, CHUNK], dt.float32, tag="pse")
                    nc.tensor.matmul(pse[:], lhsT=Wec_sb[i][:], rhs=scs[:, sl],
                                     start=True, stop=True)
                    stat_pass(pse[:], c, sse, sqe)
                    nc.scalar.activation(ea_sh[:, sl], pse[:], ACTF.Copy)

                # scatter: stream 0 = own edges (local table, no collective
                # wait), streams 1/2 = remote halves A/B; one-hot matmuls
                # accumulate into aggr
                ssa = stp.tile([F, NBLK], dt.float32, tag="ssa")
                sqa = stp.tile([F, NBLK], dt.float32, tag="sqa")
                for st in range(3):
                    scpb = cpb_o if st == 0 else cpb
                    sg2 = g2o if st == 0 else g2
                    sgc = GCALL_O if st == 0 else GCALL
                    sncalls = ncalls_o if st == 0 else ncalls_h
                    base = 0 if st == 0 else nstream_o + (st - 1) * nstream_h
                    if st == 0:
                        src_ap = htab_own[:, :]
                    else:
                        src_ap = agh_outs[i][st - 1][:, :]
                    for call in range(sncalls):
                        gt = gp.tile([128, 4 * cpb, F], dt.float16, tag="g")
                        j0 = base + call * sgc
                        # queue from a global counter: tile rotates DMASW sem
                        # lanes mod 8 per pool-DMA inst; lane L must always
                        # pair with queue L%4
                        nc.gpsimd.dma_gather(
                            out_ap=gt[:, :4 * scpb, :],
                            in_ap=src_ap,
                            idxs_ap=gidx_sb[:, j0 // 16:(j0 + sgc) // 16],
                            num_idxs=sgc, num_idxs_reg=sgc, elem_size=F,
                            queue_num=gather_seq[0] % 4,
                            single_packet=False)
                        gather_seq[0] += 1
                        for k8 in range(4 * scpb):
                            ci = call * 4 * scpb + k8   # chunk within stream
                            if ci % sg2 == 0:
                                oht = ohp.tile([128, g2, F], dt.float8e4,
                                               tag="oh")
                                if st == 0:
                                    nc.sync.dma_start(
                                        oht[:, :sg2, :], ohc.ap()[ci // sg2])
                                else:
                                    rci = (st - 1) * nch_h + ci
                                    nc.sync.dma_start(
                                        oht[:, :sg2, :], ohcr.ap()[rci // sg2])
                            b = ci // scpb
                            k = ci % scpb
                            if k == 0:
                                psb = ps_sc.tile([F, F], dt.float32, tag="sc")
                            last = (k == scpb - 1)
                            nc.tensor.matmul(psb[:], lhsT=gt[:, k8, :],
                                             rhs=oht[:, ci % sg2, :],
                                             start=(k == 0),
                                             stop=(last and st < 2))
                            if last:
                                dst = aggr_sh[:, b * F:(b + 1) * F]
                                if st == 0:
                                    nc.scalar.activation(dst, psb[:], ACTF.Copy)
                                elif st == 1:
                                    nc.vector.scalar_tensor_tensor(
                                        out=dst, in0=psb[:], scalar=1.0,
                                        in1=dst, op0=OP.mult, op1=OP.add)
                                else:
                                    # fold bnb (x) c_in into the same psum
                                    nc.tensor.matmul(
                                        psb[:], lhsT=Bnb_sb[i][:],
                                        rhs=cii[:, b * F:(b + 1) * F],
                                        start=False, stop=True)
                                    # aggr += psb ; stats
                                    nc.vector.scalar_tensor_tensor(
                                        out=dst, in0=psb[:], scalar=1.0,
                                        in1=dst, op0=OP.mult, op1=OP.add)
                                    nc.vector.tensor_reduce(
                                        ssa[:, b:b + 1], dst, AX, OP.add)
                                    tr8 = tp.tile([F, F], dt.float32, tag="tr8")
                                    nc.scalar.activation(
                                        tr8[:], dst, ACTF.Square,
                                        accum_out=sqa[:, b:b + 1])

                # AR#1: xn / ea / aggr stats
                stg = do_allreduce(1 + 3 * i, [
                    slot_reduce(ssn, NCH_SH), slot_reduce(sqn, NCH_SH),
                    slot_reduce(sse, NCH_SH), slot_reduce(sqe, NCH_SH),
                    slot_reduce(ssa, NBLK), slot_reduce(sqa, NBLK)])
                sc_n, sh_n = finalize_bn(gc_sb[f"gn{i}"], gc_sb[f"btn{i}"],
                                         stg[:, 0:1], stg[:, 1:2])
                sc_e, sh_e = finalize_bn(gc_sb[f"ge{i}"], gc_sb[f"bte{i}"],
                                         stg[:, 2:3], stg[:, 3:4])
                sc_a, sh_a = finalize_bn(gc_sb[f"gnb{i}"], gc_sb[f"btnb{i}"],
                                         stg[:, 4:5], stg[:, 5:6])
                shsum = afp.tile([F, 1], dt.float32)
                nc.vector.scalar_tensor_tensor(
                    out=shsum[:], in0=sh_n[:], scalar=1.0, in1=sh_e[:],
                    op0=OP.mult, op1=OP.add)
                nc.vector.scalar_tensor_tensor(
                    out=shsum[:], in0=shsum[:], scalar=1.0, in1=sh_a[:],
                    op0=OP.mult, op1=OP.add)

                # y1 = relu(sc_n*xn + sc_a*aggr + sc_e*ea + shsum) * mask ; m1
                ss1 = stp.tile([F, NCH_SH], dt.float32, tag="ss1")
                sq1 = stp.tile([F, NCH_SH], dt.float32, tag="sq1")
                for c in range(NCH_SH):
                    sl = slice(c * CHUNK, (c + 1) * CHUNK)
                    u1 = tp.tile([F, CHUNK], dt.float32, tag="u1")
                    nc.scalar.activation(u1[:], xn_sh[:, sl], ACTF.Copy,
                                         scale=sc_n[:])
                    u2 = tp.tile([F, CHUNK], dt.float32, tag="u2")
                    nc.vector.scalar_tensor_tensor(
                        out=u2[:], in0=aggr_sh[:, sl], scalar=sc_a[:],
                        in1=u1[:], op0=OP.mult, op1=OP.add)
                    nc.vector.scalar_tensor_tensor(
                        out=u2[:], in0=ea_sh[:, sl], scalar=sc_e[:],
                        in1=u2[:], op0=OP.mult, op1=OP.add)
                    y1 = tp.tile([F, CHUNK], dt.float16, tag="y1")
                    nc.scalar.activation(y1[:], u2[:], ACTF.Relu,
                                         bias=shsum[:], scale=1.0)
                    nc.vector.tensor_mul(y1[:], y1[:], mask_sb[:, sl])
                    ps1 = ps_mm.tile([F, CHUNK], dt.float32, tag="mm")
                    nc.tensor.matmul(ps1[:], lhsT=W1_sb[i][:], rhs=y1[:],
                                     start=True, stop=True)
                    stat_pass(ps1[:], c, ss1, sq1)
                    nc.scalar.activation(x_sh[:, sl], ps1[:], ACTF.Copy)
                stg = do_allreduce(2 + 3 * i, [slot_reduce(ss1, NCH_SH),
                                               slot_reduce(sq1, NCH_SH)])
                sc1, sh1 = finalize_bn(gc_sb[f"gm1{i}"], gc_sb[f"btm1{i}"],
                                       stg[:, 0:1], stg[:, 1:2])

                # y2 = relu(bn(m1)) * mask ; m2
                ss2 = stp.tile([F, NCH_SH], dt.float32, tag="ss2")
                sq2 = stp.tile([F, NCH_SH], dt.float32, tag="sq2")
                for c in range(NCH_SH):
                    sl = slice(c * CHUNK, (c + 1) * CHUNK)
                    y2 = tp.tile([F, CHUNK], dt.float16, tag="y2")
                    nc.scalar.activation(y2[:], x_sh[:, sl], ACTF.Relu,
                                         bias=sh1[:], scale=sc1[:])
                    nc.vector.tensor_mul(y2[:], y2[:], mask_sb[:, sl])
                    ps2 = ps_mm.tile([F, CHUNK], dt.float32, tag="mm")
                    nc.tensor.matmul(ps2[:], lhsT=W2_sb[i][:], rhs=y2[:],
                                     start=True, stop=True)
                    stat_pass(ps2[:], c, ss2, sq2)
                    nc.scalar.activation(x_sh[:, sl], ps2[:], ACTF.Copy)
                stg = do_allreduce(3 + 3 * i, [slot_reduce(ss2, NCH_SH),
                                               slot_reduce(sq2, NCH_SH)])
                sc2, sh2 = finalize_bn(gc_sb[f"gm2{i}"], gc_sb[f"btm2{i}"],
                                       stg[:, 0:1], stg[:, 1:2])

                # x_next = relu(bn(m2)) * mask (+ fp32 out on last layer)
                for c in range(NCH_SH):
                    sl = slice(c * CHUNK, (c + 1) * CHUNK)
                    if i == L - 1:
                        of = tp.tile([F, CHUNK], dt.float32, tag="of")
                        nc.scalar.activation(of[:], x_sh[:, sl], ACTF.Relu,
                                             bias=sh2[:], scale=sc2[:])
                        nc.sync.dma_start(out.ap()[:, sl], of[:])
                    else:
                        nc.scalar.activation(x_sh[:, sl], x_sh[:, sl],
                                             ACTF.Relu, bias=sh2[:],
                                             scale=sc2[:])
                        nc.vector.tensor_mul(x_sh[:, sl], x_sh[:, sl],
                                             mask_sb[:, sl])

    nc.compile()
    return nc


def kernel(**inputs):
    import sys
    for p in ("/opt/trn_rl_repo",):
        if p not in sys.path:
            sys.path.insert(0, p)
    from concourse import bass_utils

    meta = _prep(inputs["node_attr"], inputs["edge_index"], inputs["edge_attr"])

    nc = _build(meta)

    def col(v):
        return np.ascontiguousarray(v.astype(np.float32).reshape(F, 1))

    base = dict(
        W0=inputs["W0"].astype(BF16),
        g0=col(inputs["g0"]), bt0=col(inputs["bt0"]),
    )
    for i in range(L):
        base[f"Wn{i}"] = inputs["Wnode"][i].astype(BF16)
        base[f"Wb{i}"] = inputs["Wnb"][i].astype(BF16)
        base[f"W1{i}"] = inputs["Wm1"][i].astype(BF16)
        base[f"W2{i}"] = inputs["Wm2"][i].astype(BF16)
        base[f"Wec{i}"] = np.ascontiguousarray(
            np.stack([inputs["Wedge"][i][0], inputs["bedge"][i]]).astype(BF16))
        base[f"Bnb{i}"] = np.ascontiguousarray(
            inputs["bnb"][i].astype(BF16).reshape(1, F))
        for nm in ("gn", "btn", "ge", "bte", "gnb", "btnb",
                   "gm1", "btm1", "gm2", "btm2"):
            base[f"{nm}{i}"] = col(inputs[nm][i])

    in_maps = []
    for r in range(NCORES):
        m = dict(base)
        m["naT"] = meta["naT_r"][r]
        m["gidx"] = meta["gidx"][r]
        m["ohc"] = meta["ohc"][r]
        m["ohcr"] = meta["ohcr"][r]
        m["ohr"] = meta["ohr"][r]
        m["eav"] = meta["eav_t"][r]
        m["cot"] = meta["cot_r"][r]
        m["cin"] = meta["cin_r"][r]
        m["maskb"] = meta["mask_r"][r]
        in_maps.append(m)

    res = bass_utils.run_bass_kernel_spmd(
        nc, in_maps, core_ids=list(range(NCORES)))
    xT = np.concatenate([res.results[r]["out"] for r in range(NCORES)], axis=1)
    return np.ascontiguousarray(xT.T[:NREAL]).astype(np.float32)


if __name__ == "__main__":
    pass
